# revision 7
# baseline (speedup 1.0000x reference)
"""Multi-head causal attention (B=4, T=2048, D=1024, H=16, Dh=64) on 8 trn2 cores.

Sharding: 4-way DP over batch x 2-way TP over heads.
Core c handles batch c//2 and heads (c%2)*8 .. (c%2)*8+7.
Each core computes a partial output [T, D] (its heads' contribution through
w_out rows); host sums the two partials per batch.

Per-core device kernel (bf16 matmul operands, fp32 PSUM accumulation):
  v[t, f]   = sum_d xT[d, t] * w_v[d, f]      (v in [tok, feat] layout,
                                               + fused ones column per head)
  qkT[f, t] = sum_d w_qk[d, f] * xT[d, t]     (q/k in [feat, tok] layout)
  attention per (head pair hp, q-block j of 512, k-tile kt of 128):
      S^T[k, q] = sum_d kT[d, k] * qT[d, q]   (two K=64 matmuls on disjoint
                                               PE row groups -> concurrent)
      P^T = exp(S^T / 8)                      (no max-subtraction: scores ~N(0,1))
      causal mask on diagonal k-tiles via gpsimd affine_select
      o^T[m, q] = sum_k v_aug[k, m] * P^T[k, q]   (m: 64 v-feats + ones row
                                                   -> row 64 = softmax denom)
  epilogue per (hp, j): both heads' denominator rows -> one [1,1024] sbuf row,
      one reciprocal_approx_fast + bf16 cast, two rank-1 PE broadcasts into a
      single [128,512] PSUM tile (head B via tile_position col 64), then two
      DVE multiplies straight out of the PV PSUM banks into attn_t.
  y[t, n] = sum_f attn^T[f, t] * w_o[f, n]    (for the last q-block, the
      hp0-2 partial sums are computed early into SBUF and only the hp3
      quarter + an add run after the final epilogue)

Scheduling: one global "period" per attention k-tile (160 total).  All
deferred PE work (V/QK projection groups, output-projection groups) sits in
a deadline queue; each period a surplus tracker compares emitted-PE-ns
against the ACT exp cost and pops just enough work to keep the PE fed
locally (ps_st double buffering only allows ~2 periods of slack, and any
sustained PE idle re-throttles the clock to 1.2 GHz via HAM).  Epilogue
stages are spread over the three periods following each (hp, j) block.
"""

import numpy as np
import ml_dtypes

import concourse.mybir as mybir
import concourse.tile as tile
from concourse import bacc, bass_utils

F32 = mybir.dt.float32
BF16 = mybir.dt.bfloat16

D = 1024          # model dim
T = 2048          # tokens per batch
DH = 64           # head dim
NH_LOC = 8        # heads per core
DT = D // 128     # D tiles (contraction)
TT = T // 128     # token tiles
QB = T // 512     # q blocks of 512
VW = DH + 1       # v width incl ones column

JOFF = [0, 4, 12, 24]


def g_of(hp, j, kt):
    return hp * 40 + JOFF[j] + kt + 1   # global period index, 1..160


def build_kernel():
    nc = bacc.Bacc()
    xT_d = nc.dram_tensor("xT", [D, T], BF16, kind="ExternalInput")
    wqk_d = nc.dram_tensor("w_qk", [D, 1024], BF16, kind="ExternalInput")
    wv_d = nc.dram_tensor("w_v", [D, 512], BF16, kind="ExternalInput")
    wo_d = nc.dram_tensor("w_o", [512, D], BF16, kind="ExternalInput")
    y_d = nc.dram_tensor("y", [T, D], F32, kind="ExternalOutput")

    with tile.TileContext(nc) as tc:
        with (
            tc.tile_pool(name="big", bufs=1) as big,
            tc.tile_pool(name="ptp", bufs=6) as ptp,
            tc.tile_pool(name="osb", bufs=8) as osb,
            tc.tile_pool(name="ovp", bufs=2) as ovp,
            tc.tile_pool(name="stg", bufs=2) as stg,
            tc.tile_pool(name="ps_st", bufs=2, space="PSUM") as ps_st,
            tc.tile_pool(name="ps_pv", bufs=2, space="PSUM") as ps_pv,
            tc.tile_pool(name="ps_mm", bufs=2, space="PSUM") as ps_mm,
        ):
            xt_a = big.tile([128, DT, T], BF16, tag="xt", name="xt")
            wqk_a = big.tile([128, DT, 1024], BF16, tag="wqk", name="wqk")
            wv_a = big.tile([128, DT, 512], BF16, tag="wv", name="wv")
            wo_a = big.tile([128, 4, 1024], BF16, tag="wo", name="wo")
            qk = [big.tile([128, T], BF16, tag=f"qk{i}", name=f"qk{i}") for i in range(8)]
            attn_t = [big.tile([128, T], BF16, tag=f"attn{i}", name=f"attn{i}") for i in range(4)]
            vsb_t = [big.tile([128, 2, NH_LOC * VW], BF16, tag=f"vsb{i}", name=f"vsb{i}") for i in range(8)]
            ones = big.tile([1, DH], BF16, tag="ones")
            vsb_r = [t.rearrange("p t (h c) -> p t h c", c=VW) for t in vsb_t]

            # dram views with the dt tile index as an explicit dim
            xT_r = xT_d.rearrange("(i p) t -> p i t", p=128)
            wqk_r = wqk_d.rearrange("(i p) f -> p i f", p=128)
            wv_r = wv_d.rearrange("(i p) f -> p i f", p=128)
            wo_r = wo_d.rearrange("(i p) f -> p i f", p=128)

            # ---- input DMAs: one big strided DMA per wave, spread across
            # the three DMA-capable queues so wave 0 lands fast ----
            nc.gpsimd.dma_start(wv_a, wv_r)
            nc.sync.dma_start(xt_a[:, :, 0:512], xT_r[:, :, 0:512])
            for f in (0, 4):  # head-pair 0's q/k weight columns
                nc.scalar.dma_start(
                    wqk_a[:, :, f * 128:(f + 1) * 128],
                    wqk_r[:, :, f * 128:(f + 1) * 128],
                )
            for tb in range(1, QB):
                nc.sync.dma_start(
                    xt_a[:, :, tb * 512:(tb + 1) * 512],
                    xT_r[:, :, tb * 512:(tb + 1) * 512],
                )
            for f in (1, 5, 2, 6, 3, 7):  # in consumer (head-pair) order
                nc.gpsimd.dma_start(
                    wqk_a[:, :, f * 128:(f + 1) * 128],
                    wqk_r[:, :, f * 128:(f + 1) * 128],
                )
            nc.gpsimd.dma_start(wo_a, wo_r)
            nc.vector.memset(ones, 1.0)

            # ---- projection / output-projection group emitters ----
            def v_group(tt):
                def go():
                    ps = ps_mm.tile([128, 512], F32, tag="mm")
                    for dt in range(DT):
                        nc.tensor.matmul(
                            ps,
                            lhsT=xt_a[:, dt, tt * 128:(tt + 1) * 128],
                            rhs=wv_a[:, dt, :],
                            start=(dt == 0),
                            stop=(dt == DT - 1),
                        )
                    nc.vector.tensor_copy(
                        vsb_r[tt // 2][:, tt % 2, :, 0:DH],
                        ps.rearrange("p (h c) -> p h c", c=DH),
                    )
                    nc.vector.memset(vsb_r[tt // 2][:, tt % 2, :, DH], 1.0)
                return go

            def qk_group(f, tb):
                def go():
                    ps = ps_mm.tile([128, 512], F32, tag="mm")
                    for dt in range(DT):
                        nc.tensor.matmul(
                            ps,
                            lhsT=wqk_a[:, dt, f * 128:(f + 1) * 128],
                            rhs=xt_a[:, dt, tb * 512:(tb + 1) * 512],
                            start=(dt == 0),
                            stop=(dt == DT - 1),
                        )
                    nc.vector.tensor_copy(qk[f][:, tb * 512:(tb + 1) * 512], ps)
                return go

            def out_group(tt, nb):
                def go():
                    ps = ps_mm.tile([128, 512], F32, tag="mm")
                    for hp4 in range(4):
                        nc.tensor.matmul(
                            ps,
                            lhsT=attn_t[hp4][:, tt * 128:(tt + 1) * 128],
                            rhs=wo_a[:, hp4, nb * 512:(nb + 1) * 512],
                            start=(hp4 == 0),
                            stop=(hp4 == 3),
                        )
                    ysb = stg.tile([128, 512], F32, tag="y", bufs=4,
                                   name=f"ysb{tt}_{nb}")
                    nc.vector.tensor_copy(ysb, ps)
                    nc.sync.dma_start(
                        y_d[tt * 128:(tt + 1) * 128, nb * 512:(nb + 1) * 512],
                        ysb,
                    )
                return go

            # last q-block: hp0-2 partial sums early, hp3 quarter at the end
            j3_parts = {}

            def out3_partial(tt, nb):
                def go():
                    ps = ps_mm.tile([128, 512], F32, tag="mm")
                    for hp4 in range(3):
                        nc.tensor.matmul(
                            ps,
                            lhsT=attn_t[hp4][:, tt * 128:(tt + 1) * 128],
                            rhs=wo_a[:, hp4, nb * 512:(nb + 1) * 512],
                            start=(hp4 == 0),
                            stop=(hp4 == 2),
                        )
                    po = osb.tile([128, 512], F32, tag="osb",
                                  name=f"osb{tt}_{nb}")
                    nc.vector.tensor_copy(po, ps)
                    j3_parts[(tt, nb)] = po
                return go

            def out3_final(tt, nb):
                ps = ps_mm.tile([128, 512], F32, tag="mm")
                nc.tensor.matmul(
                    ps,
                    lhsT=attn_t[3][:, tt * 128:(tt + 1) * 128],
                    rhs=wo_a[:, 3, nb * 512:(nb + 1) * 512],
                    start=True, stop=True,
                )
                ysb = stg.tile([128, 512], F32, tag="y", bufs=4,
                               name=f"yf{tt}_{nb}")
                nc.vector.tensor_add(ysb, j3_parts[(tt, nb)], ps)
                nc.sync.dma_start(
                    y_d[tt * 128:(tt + 1) * 128, nb * 512:(nb + 1) * 512],
                    ysb,
                )

            # ---- deadline + surplus driven scheduler ----
            GROUP_NS = 1730.0   # 8 N=512 matmuls
            OUT_NS = 870.0      # 4 N=512 matmuls
            OUT3_NS = 660.0     # 3 N=512 matmuls
            MARGIN = 800.0
            CAP = 2600.0
            FLOOR = -2600.0
            SAFE = 3

            units = []   # dicts: avail, deadline, cost, fn
            seq = [0]

            def add_unit(avail, deadline, cost, fn):
                units.append(
                    {"avail": avail, "dl": deadline, "cost": cost,
                     "fn": fn, "seq": seq[0]}
                )
                seq[0] += 1

            # V tiles: used by PV kt=tt of hp0 block j=tt//4 (2-period delay)
            for tt in range(4, 16):
                add_unit(0, g_of(0, tt // 4, tt) + 2 - SAFE, GROUP_NS, v_group(tt))
            # QK halves: q half f=p used from (p, tb, 0); k half f=4+p from
            # (p, tb, 4*tb)
            for p in range(4):
                for tb in range(QB):
                    if p == 0 and tb == 0:
                        continue
                    add_unit(0, g_of(p, tb, 0) - SAFE, GROUP_NS, qk_group(p, tb))
                    add_unit(0, g_of(p, tb, 4 * tb) - SAFE, GROUP_NS,
                             qk_group(4 + p, tb))

            sched = {}   # period -> list of (pe_ns, closure)

            def at(g, pe_ns, fn):
                sched.setdefault(g, []).append((pe_ns, fn))

            state = {"surplus": float(CAP)}

            def tick(g, act_ns, pe_ns):
                for extra_pe, fn in sched.pop(g, ()):
                    fn()
                    pe_ns += extra_pe
                s = state["surplus"] + pe_ns - act_ns
                # forced (deadline-due) units
                rest = []
                for u in units:
                    if u["avail"] <= g and u["dl"] <= g:
                        u["fn"]()
                        s += u["cost"]
                    else:
                        rest.append(u)
                units[:] = rest
                # greedy: keep a small PE backlog
                while s < MARGIN:
                    ready = [u for u in units if u["avail"] <= g]
                    if not ready:
                        break
                    u = min(ready, key=lambda u: (u["dl"], u["seq"]))
                    units.remove(u)
                    u["fn"]()
                    s += u["cost"]
                state["surplus"] = min(CAP, max(FLOOR, s))

            # ---- epilogue: one recip row for both heads, PE broadcast,
            # DVE multiply straight out of the PV PSUM banks ----
            def push_epilogue(hp, j, pvA, pvB, inline=False):
                g0 = g_of(hp, j, 4 * (j + 1) - 1)
                dn1 = stg.tile([1, 1024], F32, tag="dn", name=f"dn{hp}_{j}")
                rec1 = stg.tile([1, 1024], F32, tag="rec", name=f"rec{hp}_{j}")
                rb1 = stg.tile([1, 1024], BF16, tag="rb", name=f"rb{hp}_{j}")

                def stage_ab():
                    nc.vector.tensor_copy(dn1[0:1, 0:512], pvA[DH:DH + 1, :])
                    nc.vector.tensor_copy(dn1[0:1, 512:1024], pvB[DH:DH + 1, :])
                    nc.vector.reciprocal_approx_fast(out=rec1, in_=dn1)
                    nc.vector.tensor_copy(rb1, rec1)

                def stage_cd():
                    ov = ovp.tile([128, 512], F32, tag="ov", name=f"ov{hp}_{j}")
                    nc.vector.tensor_copy(ov[0:DH, :], pvA[0:DH, :])
                    nc.vector.tensor_copy(ov[DH:128, :], pvB[0:DH, :])
                    bc = ps_mm.tile([128, 512], F32, tag="mm")
                    nc.tensor.matmul(bc[0:DH, :], lhsT=ones,
                                     rhs=rb1[0:1, 0:512], start=True, stop=True)
                    nc.tensor.matmul(bc[DH:128, :], lhsT=ones,
                                     rhs=rb1[0:1, 512:1024], start=True,
                                     stop=True, tile_position=(0, 64))
                    stage_cd.ov = ov
                    stage_cd.bc = bc

                def stage_e():
                    jc = slice(j * 512, (j + 1) * 512)
                    nc.vector.tensor_mul(
                        attn_t[hp][:, jc], stage_cd.ov, stage_cd.bc
                    )

                if inline:
                    stage_ab(); stage_cd(); stage_e()
                else:
                    at(g0 + 1, 0.0, stage_ab)
                    at(g0 + 2, 450.0, stage_cd)
                    at(g0 + 3, 0.0, stage_e)

            # ---- prologue: exactly what attention block (pair0, j=0) needs
            for tt in range(4):
                v_group(tt)()
            qk_group(0, 0)()
            qk_group(4, 0)()

            # ---- attention: head-PAIR outer, j inner, one k-tile per period
            for hp in range(4):
                qTf = qk[hp]
                kTf = qk[4 + hp]
                for j in range(QB):
                    pvA = ps_pv.tile([128, 512], F32, tag="pv")
                    pvB = ps_pv.tile([128, 512], F32, tag="pv")
                    nkt = 4 * (j + 1)
                    pv_queue = []  # PV MMs delayed 2 periods behind S^T/exp
                    for kt in range(nkt):
                        g = g_of(hp, j, kt)
                        # diagonal k-tiles: q < 128*(kt-4j) is fully masked --
                        # narrow S^T/exp/mask/PV to the live columns
                        q0 = 128 * (kt - 4 * j) if kt >= 4 * j else 0
                        nq = 512 - q0
                        st = ps_st.tile([128, 1024], F32, tag="st")
                        nc.tensor.matmul(
                            st[:, q0:512],
                            lhsT=kTf[0:64, kt * 128:(kt + 1) * 128],
                            rhs=qTf[0:64, j * 512 + q0:(j + 1) * 512],
                            start=True, stop=True,
                        )
                        nc.tensor.matmul(
                            st[:, 512 + q0:1024],
                            lhsT=kTf[64:128, kt * 128:(kt + 1) * 128],
                            rhs=qTf[64:128, j * 512 + q0:(j + 1) * 512],
                            start=True, stop=True,
                        )
                        pt = ptp.tile([128, 1024], BF16, tag="pt",
                                      name=f"pt{hp}_{j}_{kt}")
                        st_r = st.rearrange("p (h q) -> p h q", h=2)
                        pt_r = pt.rearrange("p (h q) -> p h q", h=2)
                        nc.scalar.activation(
                            pt_r[:, :, q0:512], st_r[:, :, q0:512],
                            mybir.ActivationFunctionType.Exp, scale=0.125
                        )
                        if kt >= 4 * j:  # diagonal k-tile: zero where k > q
                            for half in range(2):
                                nc.gpsimd.affine_select(
                                    out=pt[:, half * 512 + q0:(half + 1) * 512],
                                    in_=pt[:, half * 512 + q0:(half + 1) * 512],
                                    compare_op=mybir.AluOpType.is_ge,
                                    fill=0.0,
                                    base=0,
                                    pattern=[[1, nq]],
                                    channel_multiplier=-1,
                                )

                        pe_ns = nq / 2.4 + 120.0
                        if len(pv_queue) >= 2:
                            nqp, f_ = pv_queue.pop(0)
                            f_()
                            pe_ns += 2 * nqp / 2.4 + 100.0

                        def pv_mms(kt=kt, pt=pt, q0=q0):
                            nc.tensor.matmul(
                                pvA[0:VW, q0:512],
                                lhsT=vsb_r[kt // 2][:, kt % 2, 2 * hp, :],
                                rhs=pt[:, q0:512],
                                start=(kt == 0), stop=(kt == nkt - 1),
                            )
                            nc.tensor.matmul(
                                pvB[0:VW, q0:512],
                                lhsT=vsb_r[kt // 2][:, kt % 2, 2 * hp + 1, :],
                                rhs=pt[:, 512 + q0:1024],
                                start=(kt == 0), stop=(kt == nkt - 1),
                            )
                        pv_queue.append((nq, pv_mms))
                        tick(g, (2 * nq + 352) / 1.2, pe_ns)
                    for nqp, f_ in pv_queue:
                        f_()
                        state["surplus"] = min(CAP, state["surplus"] + 2 * nqp / 2.4)
                    push_epilogue(hp, j, pvA, pvB,
                                  inline=(hp == 3 and j == QB - 1))
                    if hp == 3 and j < QB - 1:
                        # out-proj for q-block j: available once stage_e lands
                        g_av = g_of(3, j, 4 * (j + 1) - 1) + 4
                        for tt in range(4 * j, 4 * j + 4):
                            for nb in range(2):
                                add_unit(g_av, g_av + 8, OUT_NS, out_group(tt, nb))
                    if hp == 2 and j == QB - 1:
                        # hp0-2 partials of the last q-block's out-proj
                        g_av = g_of(2, 3, 15) + 4
                        for tt in range(12, 16):
                            for nb in range(2):
                                add_unit(g_av, g_av + 16, OUT3_NS,
                                         out3_partial(tt, nb))

            # ---- drain: leftover scheduled stages, units, then the last
            # q-block's final out-proj quarter ----
            for g in sorted(sched.keys()):
                for _, fn in sched.pop(g):
                    fn()
            for u in sorted(units, key=lambda u: (u["dl"], u["seq"])):
                u["fn"]()
            units[:] = []
            for tt in range(12, 16):
                for nb in range(2):
                    out3_final(tt, nb)

    nc.compile()
    return nc


def _shard_inputs(x, w_qkv, w_out):
    """Build the 8 per-core input maps (matmul operands pre-cast to bf16)."""
    bf16 = ml_dtypes.bfloat16
    in_maps = []
    for c in range(8):
        b = c // 2
        hg = c % 2
        q_cols = slice(hg * 512, hg * 512 + 512)
        k_cols = slice(1024 + hg * 512, 1024 + hg * 512 + 512)
        v_cols = slice(2048 + hg * 512, 2048 + hg * 512 + 512)
        in_maps.append({
            "xT": np.ascontiguousarray(x[b].T).astype(bf16),
            "w_qk": np.ascontiguousarray(
                np.concatenate([w_qkv[:, q_cols], w_qkv[:, k_cols]], axis=1)
            ).astype(bf16),
            "w_v": np.ascontiguousarray(w_qkv[:, v_cols]).astype(bf16),
            "w_o": np.ascontiguousarray(w_out[hg * 512:hg * 512 + 512, :]).astype(bf16),
        })
    return in_maps


def _run(inputs, trace=False):
    x = np.asarray(inputs["x"], dtype=np.float32)
    w_qkv = np.asarray(inputs["w_qkv"], dtype=np.float32)
    w_out = np.asarray(inputs["w_out"], dtype=np.float32)
    nc = build_kernel()
    in_maps = _shard_inputs(x, w_qkv, w_out)
    res = None
    for attempt in range(3):
        try:
            res = bass_utils.run_bass_kernel_spmd(
                nc, in_maps, core_ids=list(range(8)), trace=trace
            )
            break
        except Exception:
            if attempt == 2:
                raise
    assert res is not None
    out = np.empty((4, T, D), dtype=np.float32)
    for b in range(4):
        out[b] = res.results[2 * b]["y"] + res.results[2 * b + 1]["y"]
    return out, res


def kernel(**inputs):
    out, _ = _run(inputs, trace=False)
    return out


# revision 12
# speedup vs baseline: 1.0597x; 1.0597x over previous
"""Multi-head causal attention (B=4, T=2048, D=1024, H=16, Dh=64) on 8 trn2 cores.

Sharding: 4-way DP over batch x 2-way TP over heads.
Core c handles batch c//2 and heads (c%2)*8 .. (c%2)*8+7.
Each core computes a partial output [T, D] (its heads' contribution through
w_out rows); host sums the two partials per batch.

Per-core device kernel (bf16 matmul operands, fp32 PSUM accumulation):
  v[t, f]   = sum_d xT[d, t] * w_v[d, f]      (v in [tok, feat] layout,
                                               + fused ones column per head)
  qkT[f, t] = sum_d w_qk[d, f] * xT[d, t]     (q/k in [feat, tok] layout)
  attention per (head pair hp, q-block j of 512, k-tile kt of 128):
      S^T[k, q] = sum_d kT[d, k] * qT[d, q]   (two K=64 matmuls on disjoint
                                               PE row groups -> concurrent)
      P^T = exp(S^T / 8)                      (no max-subtraction: scores ~N(0,1))
      causal mask on diagonal k-tiles via gpsimd affine_select
      o^T[m, q] = sum_k v_aug[k, m] * P^T[k, q]   (m: 64 v-feats + ones row
                                                   -> row 64 = softmax denom)
  epilogue per (hp, j): both heads' denominator rows -> one [1,1024] sbuf row,
      one reciprocal_approx_fast + bf16 cast, two rank-1 PE broadcasts into a
      single [128,512] PSUM tile (head B via tile_position col 64), then two
      DVE multiplies straight out of the PV PSUM banks into attn_t.
  y[t, n] = sum_f attn^T[f, t] * w_o[f, n]    (for the last q-block, the
      hp0-2 partial sums are computed early into SBUF and only the hp3
      quarter + an add run after the final epilogue)

Scheduling: one global "period" per attention k-tile (160 total).  All
deferred PE work (V/QK projection groups, output-projection groups) sits in
a deadline queue; each period a surplus tracker compares emitted-PE-ns
against the ACT exp cost and pops just enough work to keep the PE fed
locally (ps_st double buffering only allows ~2 periods of slack, and any
sustained PE idle re-throttles the clock to 1.2 GHz via HAM).  Epilogue
stages are spread over the three periods following each (hp, j) block.
"""

import numpy as np
import ml_dtypes

import concourse.mybir as mybir
import concourse.tile as tile
from concourse import bacc, bass_utils

F32 = mybir.dt.float32
BF16 = mybir.dt.bfloat16

D = 1024          # model dim
T = 2048          # tokens per batch
DH = 64           # head dim
NH_LOC = 8        # heads per core
DT = D // 128     # D tiles (contraction)
TT = T // 128     # token tiles
QB = T // 512     # q blocks of 512
VW = DH + 1       # v width incl ones column

JOFF = [0, 4, 12, 24]


def g_of(hp, j, kt):
    return hp * 40 + JOFF[j] + kt + 1   # global period index, 1..160


def build_kernel():
    nc = bacc.Bacc()
    xT_d = nc.dram_tensor("xT", [D, T], BF16, kind="ExternalInput")
    wqk_d = nc.dram_tensor("w_qk", [D, 1024], BF16, kind="ExternalInput")
    wv_d = nc.dram_tensor("w_v", [D, 512], BF16, kind="ExternalInput")
    wo_d = nc.dram_tensor("w_o", [512, D], BF16, kind="ExternalInput")
    y_d = nc.dram_tensor("y", [T, D], F32, kind="ExternalOutput")

    with tile.TileContext(nc) as tc:
        with (
            tc.tile_pool(name="big", bufs=1) as big,
            tc.tile_pool(name="ptp", bufs=6) as ptp,
            tc.tile_pool(name="osb", bufs=1) as osb,
            tc.tile_pool(name="ovp", bufs=2) as ovp,
            tc.tile_pool(name="stg", bufs=2) as stg,
            tc.tile_pool(name="ps_st", bufs=2, space="PSUM") as ps_st,
            tc.tile_pool(name="ps_pv", bufs=2, space="PSUM") as ps_pv,
            tc.tile_pool(name="ps_mm", bufs=2, space="PSUM") as ps_mm,
        ):
            xt_a = big.tile([128, DT, T], BF16, tag="xt", name="xt")
            wqk_a = big.tile([128, DT, 1024], BF16, tag="wqk", name="wqk")
            wv_a = big.tile([128, DT, 512], BF16, tag="wv", name="wv")
            wo_a = big.tile([128, 4, 1024], BF16, tag="wo", name="wo")
            qk = [big.tile([128, T], BF16, tag=f"qk{i}", name=f"qk{i}") for i in range(8)]
            attn_t = [big.tile([128, T], BF16, tag=f"attn{i}", name=f"attn{i}") for i in range(4)]
            vsb_t = [big.tile([128, 2, NH_LOC * VW], BF16, tag=f"vsb{i}", name=f"vsb{i}") for i in range(8)]
            ones = big.tile([1, DH], BF16, tag="ones")
            vsb_r = [t.rearrange("p t (h c) -> p t h c", c=VW) for t in vsb_t]

            # dram views with the dt tile index as an explicit dim
            xT_r = xT_d.rearrange("(i p) t -> p i t", p=128)
            wqk_r = wqk_d.rearrange("(i p) f -> p i f", p=128)
            wv_r = wv_d.rearrange("(i p) f -> p i f", p=128)
            wo_r = wo_d.rearrange("(i p) f -> p i f", p=128)

            # ---- input DMAs: per-dt chunks spread across the three
            # DMA-capable queues (many parallel DMA engines, and each
            # accumulation step only waits for its own chunk) ----
            rr = [nc.sync, nc.scalar, nc.gpsimd]
            for i in range(DT):
                rr[i % 3].dma_start(wv_a[:, i, :], wv_r[:, i, :])
                rr[(i + 1) % 3].dma_start(xt_a[:, i, 0:512], xT_r[:, i, 0:512])
            for f in (0, 4):  # head-pair 0's q/k weight columns
                nc.scalar.dma_start(
                    wqk_a[:, :, f * 128:(f + 1) * 128],
                    wqk_r[:, :, f * 128:(f + 1) * 128],
                )
            # x token-blocks 1-3 chunked on sync+gpsimd (gpsimd's portion
            # issues before the first affine_selects need that queue)
            rr2 = [nc.sync, nc.gpsimd]
            for tb in range(1, QB):
                for i in range(DT):
                    rr2[(tb * DT + i) % 2].dma_start(
                        xt_a[:, i, tb * 512:(tb + 1) * 512],
                        xT_r[:, i, tb * 512:(tb + 1) * 512],
                    )
            for f in (1, 5, 2, 6, 3, 7):  # in consumer (head-pair) order
                nc.sync.dma_start(
                    wqk_a[:, :, f * 128:(f + 1) * 128],
                    wqk_r[:, :, f * 128:(f + 1) * 128],
                )
            nc.sync.dma_start(wo_a, wo_r)
            nc.vector.memset(ones, 1.0)

            # ---- projection / output-projection group emitters ----
            def v_group(tt):
                def go():
                    ps = ps_mm.tile([128, 512], F32, tag="mm")
                    for dt in range(DT):
                        nc.tensor.matmul(
                            ps,
                            lhsT=xt_a[:, dt, tt * 128:(tt + 1) * 128],
                            rhs=wv_a[:, dt, :],
                            start=(dt == 0),
                            stop=(dt == DT - 1),
                        )
                    nc.vector.tensor_copy(
                        vsb_r[tt // 2][:, tt % 2, :, 0:DH],
                        ps.rearrange("p (h c) -> p h c", c=DH),
                    )
                    nc.vector.memset(vsb_r[tt // 2][:, tt % 2, :, DH], 1.0)
                return go

            def qk_group(f, tb):
                def go():
                    ps = ps_mm.tile([128, 512], F32, tag="mm")
                    for dt in range(DT):
                        nc.tensor.matmul(
                            ps,
                            lhsT=wqk_a[:, dt, f * 128:(f + 1) * 128],
                            rhs=xt_a[:, dt, tb * 512:(tb + 1) * 512],
                            start=(dt == 0),
                            stop=(dt == DT - 1),
                        )
                    nc.vector.tensor_copy(qk[f][:, tb * 512:(tb + 1) * 512], ps)
                return go

            # out-projection for token tile tt, out half nb, split in two:
            # partA = hp0+hp1 quarters -> fp16 SBUF partial (available as
            # soon as hp1's epilogue for that q-block lands -> PE filler for
            # the middle head-pairs); partB = hp2+hp3 quarters + add + DMA.
            F16 = mybir.dt.float16
            out_parts = {}

            def out_partA(tt, nb):
                def go():
                    ps = ps_mm.tile([128, 512], F32, tag="mm")
                    for hp4 in range(2):
                        nc.tensor.matmul(
                            ps,
                            lhsT=attn_t[hp4][:, tt * 128:(tt + 1) * 128],
                            rhs=wo_a[:, hp4, nb * 512:(nb + 1) * 512],
                            start=(hp4 == 0),
                            stop=(hp4 == 1),
                        )
                    po = osb.tile([128, 512], F16, tag=f"osb{tt}_{nb}",
                                  name=f"osb{tt}_{nb}")
                    nc.vector.tensor_copy(po, ps)
                    out_parts[(tt, nb)] = po
                return go

            def out_partB(tt, nb):
                def go():
                    ps = ps_mm.tile([128, 512], F32, tag="mm")
                    for hp4 in (2, 3):
                        nc.tensor.matmul(
                            ps,
                            lhsT=attn_t[hp4][:, tt * 128:(tt + 1) * 128],
                            rhs=wo_a[:, hp4, nb * 512:(nb + 1) * 512],
                            start=(hp4 == 2),
                            stop=(hp4 == 3),
                        )
                    ysb = stg.tile([128, 512], F32, tag="y", bufs=4,
                                   name=f"ysb{tt}_{nb}")
                    nc.vector.tensor_add(ysb, out_parts[(tt, nb)], ps)
                    nc.sync.dma_start(
                        y_d[tt * 128:(tt + 1) * 128, nb * 512:(nb + 1) * 512],
                        ysb,
                    )
                return go

            # ---- deadline + surplus driven scheduler ----
            GROUP_NS = 1730.0   # 8 N=512 matmuls
            HALF_NS = 440.0     # 2 N=512 matmuls
            MARGIN = 900.0
            CAP = 1800.0
            FLOOR = -2600.0
            SAFE = 5

            units = []   # dicts: avail, deadline, cost, fn
            seq = [0]

            def add_unit(avail, deadline, cost, fn):
                units.append(
                    {"avail": avail, "dl": deadline, "cost": cost,
                     "fn": fn, "seq": seq[0]}
                )
                seq[0] += 1

            # V tiles: used by PV kt=tt of hp0 block j=tt//4 (2-period delay)
            for tt in range(4, 16):
                add_unit(0, g_of(0, tt // 4, tt) + 2 - SAFE, GROUP_NS, v_group(tt))
            # QK halves: q half f=p used from (p, tb, 0); k half f=4+p from
            # (p, tb, 4*tb)
            for p in range(4):
                for tb in range(QB):
                    if p == 0 and tb == 0:
                        continue
                    add_unit(0, g_of(p, tb, 0) - SAFE, GROUP_NS, qk_group(p, tb))
                    add_unit(0, g_of(p, tb, 4 * tb) - SAFE, GROUP_NS,
                             qk_group(4 + p, tb))

            sched = {}   # period -> list of (pe_ns, closure)

            def at(g, pe_ns, fn):
                sched.setdefault(g, []).append((pe_ns, fn))

            state = {"surplus": float(CAP)}

            def tick(g, act_ns, pe_ns):
                for extra_pe, fn in sched.pop(g, ()):
                    fn()
                    pe_ns += extra_pe
                s = state["surplus"] + pe_ns - act_ns
                # forced (deadline-due) units, earliest deadline first
                due = [u for u in units if u["avail"] <= g and u["dl"] <= g]
                due.sort(key=lambda u: (u["dl"], u["seq"]))
                for u in due:
                    units.remove(u)
                    u["fn"]()
                    s += u["cost"]
                # greedy: keep a small PE backlog (drain eagerly in hp3)
                margin = 1400.0 if g > 120 else MARGIN
                while s < margin:
                    ready = [u for u in units if u["avail"] <= g]
                    if not ready:
                        break
                    u = min(ready, key=lambda u: (u["dl"], u["seq"]))
                    units.remove(u)
                    u["fn"]()
                    s += u["cost"]
                state["surplus"] = min(CAP, max(FLOOR, s))

            # ---- epilogue: one recip row for both heads, PE broadcast,
            # DVE multiply straight out of the PV PSUM banks ----
            def push_epilogue(hp, j, pvA, pvB):
                g0 = g_of(hp, j, 4 * (j + 1) - 1)
                dn1 = stg.tile([1, 1024], F32, tag="dn", name=f"dn{hp}_{j}")
                rec1 = stg.tile([1, 1024], F32, tag="rec", name=f"rec{hp}_{j}")
                rb1 = stg.tile([1, 1024], BF16, tag="rb", name=f"rb{hp}_{j}")

                def stage_ab():
                    nc.vector.tensor_copy(dn1[0:1, 0:512], pvA[DH:DH + 1, :])
                    nc.vector.tensor_copy(dn1[0:1, 512:1024], pvB[DH:DH + 1, :])
                    nc.vector.reciprocal_approx_fast(out=rec1, in_=dn1)
                    nc.vector.tensor_copy(rb1, rec1)

                def stage_cde():
                    ov = ovp.tile([128, 512], F32, tag="ov", name=f"ov{hp}_{j}")
                    nc.vector.tensor_copy(ov[0:DH, :], pvA[0:DH, :])
                    nc.vector.tensor_copy(ov[DH:128, :], pvB[0:DH, :])
                    bc = ps_mm.tile([128, 512], F32, tag="mm")
                    nc.tensor.matmul(bc[0:DH, :], lhsT=ones,
                                     rhs=rb1[0:1, 0:512], start=True, stop=True)
                    nc.tensor.matmul(bc[DH:128, :], lhsT=ones,
                                     rhs=rb1[0:1, 512:1024], start=True,
                                     stop=True, tile_position=(0, 64))
                    jc = slice(j * 512, (j + 1) * 512)
                    nc.vector.tensor_mul(attn_t[hp][:, jc], ov, bc)

                at(g0 + 1, 0.0, stage_ab)
                at(g0 + 2, 450.0, stage_cde)

            # ---- prologue: exactly what attention block (pair0, j=0) needs
            for tt in range(4):
                v_group(tt)()
            qk_group(0, 0)()
            qk_group(4, 0)()

            # ---- attention: head-PAIR outer, j inner, one k-tile per period
            for hp in range(4):
                qTf = qk[hp]
                kTf = qk[4 + hp]
                for j in range(QB):
                    pvA = ps_pv.tile([128, 512], F32, tag="pv")
                    pvB = ps_pv.tile([128, 512], F32, tag="pv")
                    nkt = 4 * (j + 1)
                    pv_queue = []  # PV MMs delayed 2 periods behind S^T/exp
                    for kt in range(nkt):
                        g = g_of(hp, j, kt)
                        # diagonal k-tiles: q < 128*(kt-4j) is fully masked --
                        # narrow S^T/exp/mask/PV to the live columns
                        q0 = 128 * (kt - 4 * j) if kt >= 4 * j else 0
                        nq = 512 - q0
                        st = ps_st.tile([128, 1024], F32, tag="st")
                        nc.tensor.matmul(
                            st[:, q0:512],
                            lhsT=kTf[0:64, kt * 128:(kt + 1) * 128],
                            rhs=qTf[0:64, j * 512 + q0:(j + 1) * 512],
                            start=True, stop=True,
                        )
                        nc.tensor.matmul(
                            st[:, 512 + q0:1024],
                            lhsT=kTf[64:128, kt * 128:(kt + 1) * 128],
                            rhs=qTf[64:128, j * 512 + q0:(j + 1) * 512],
                            start=True, stop=True,
                        )
                        pt = ptp.tile([128, 1024], BF16, tag="pt",
                                      name=f"pt{hp}_{j}_{kt}")
                        st_r = st.rearrange("p (h q) -> p h q", h=2)
                        pt_r = pt.rearrange("p (h q) -> p h q", h=2)
                        nc.scalar.activation(
                            pt_r[:, :, q0:512], st_r[:, :, q0:512],
                            mybir.ActivationFunctionType.Exp, scale=0.125
                        )
                        if kt >= 4 * j:  # diagonal k-tile: zero where k > q
                            for half in range(2):
                                nc.gpsimd.affine_select(
                                    out=pt[:, half * 512 + q0:(half + 1) * 512],
                                    in_=pt[:, half * 512 + q0:(half + 1) * 512],
                                    compare_op=mybir.AluOpType.is_ge,
                                    fill=0.0,
                                    base=0,
                                    pattern=[[1, nq]],
                                    channel_multiplier=-1,
                                )

                        pe_ns = nq / 2.4 + 120.0
                        if len(pv_queue) >= 2:
                            nqp, f_ = pv_queue.pop(0)
                            f_()
                            pe_ns += 2 * nqp / 2.4 + 100.0

                        def pv_mms(kt=kt, pt=pt, q0=q0):
                            nc.tensor.matmul(
                                pvA[0:VW, q0:512],
                                lhsT=vsb_r[kt // 2][:, kt % 2, 2 * hp, :],
                                rhs=pt[:, q0:512],
                                start=(kt == 0), stop=(kt == nkt - 1),
                            )
                            nc.tensor.matmul(
                                pvB[0:VW, q0:512],
                                lhsT=vsb_r[kt // 2][:, kt % 2, 2 * hp + 1, :],
                                rhs=pt[:, 512 + q0:1024],
                                start=(kt == 0), stop=(kt == nkt - 1),
                            )
                        pv_queue.append((nq, pv_mms))
                        tick(g, (2 * nq + 352) / 1.2, pe_ns)
                    for nqp, f_ in pv_queue:
                        f_()
                        state["surplus"] = min(CAP, state["surplus"] + 2 * nqp / 2.4)
                    push_epilogue(hp, j, pvA, pvB)
                    g_av = g_of(hp, j, 4 * (j + 1) - 1) + 3
                    if hp == 1:
                        # hp0+hp1 halves of this q-block's out-proj become
                        # PE filler for the middle head-pairs; must land
                        # before the matching partB can be popped
                        dlA = g_of(3, j, 4 * (j + 1) - 1) + 1
                        for tt in range(4 * j, 4 * j + 4):
                            for nb in range(2):
                                add_unit(g_av, dlA, HALF_NS,
                                         out_partA(tt, nb))
                    if hp == 3 and j < QB - 1:
                        for tt in range(4 * j, 4 * j + 4):
                            for nb in range(2):
                                add_unit(g_av, g_av + 10, HALF_NS,
                                         out_partB(tt, nb))

            # ---- drain: leftover units first (warm PE work overlapping the
            # final inline epilogue's DVE chain), then leftover stages, then
            # the last q-block's partB out-proj ----
            for u in sorted(units, key=lambda u: (u["dl"], u["seq"])):
                u["fn"]()
            units[:] = []
            for g in sorted(sched.keys()):
                for _, fn in sched.pop(g):
                    fn()
            for tt in range(12, 16):
                for nb in range(2):
                    out_partB(tt, nb)()

    nc.compile()
    return nc


def _shard_inputs(x, w_qkv, w_out):
    """Build the 8 per-core input maps (matmul operands pre-cast to bf16)."""
    bf16 = ml_dtypes.bfloat16
    in_maps = []
    for c in range(8):
        b = c // 2
        hg = c % 2
        q_cols = slice(hg * 512, hg * 512 + 512)
        k_cols = slice(1024 + hg * 512, 1024 + hg * 512 + 512)
        v_cols = slice(2048 + hg * 512, 2048 + hg * 512 + 512)
        in_maps.append({
            "xT": np.ascontiguousarray(x[b].T).astype(bf16),
            "w_qk": np.ascontiguousarray(
                np.concatenate([w_qkv[:, q_cols], w_qkv[:, k_cols]], axis=1)
            ).astype(bf16),
            "w_v": np.ascontiguousarray(w_qkv[:, v_cols]).astype(bf16),
            "w_o": np.ascontiguousarray(w_out[hg * 512:hg * 512 + 512, :]).astype(bf16),
        })
    return in_maps


def _run(inputs, trace=False):
    x = np.asarray(inputs["x"], dtype=np.float32)
    w_qkv = np.asarray(inputs["w_qkv"], dtype=np.float32)
    w_out = np.asarray(inputs["w_out"], dtype=np.float32)
    nc = build_kernel()
    in_maps = _shard_inputs(x, w_qkv, w_out)
    res = None
    for attempt in range(3):
        try:
            res = bass_utils.run_bass_kernel_spmd(
                nc, in_maps, core_ids=list(range(8)), trace=trace
            )
            break
        except Exception:
            if attempt == 2:
                raise
    assert res is not None
    out = np.empty((4, T, D), dtype=np.float32)
    for b in range(4):
        out[b] = res.results[2 * b]["y"] + res.results[2 * b + 1]["y"]
    return out, res


def kernel(**inputs):
    out, _ = _run(inputs, trace=False)
    return out


# revision 17
# speedup vs baseline: 1.0915x; 1.0300x over previous
"""Multi-head causal attention (B=4, T=2048, D=1024, H=16, Dh=64) on 8 trn2 cores.

Sharding: 4-way DP over batch x 2-way TP over heads.
Core c handles batch c//2 and heads (c%2)*8 .. (c%2)*8+7.
Each core computes a partial output [T, D] (its heads' contribution through
w_out rows); host sums the two partials per batch.

Per-core device kernel (bf16 matmul operands, fp32 PSUM accumulation):
  v[t, f]   = sum_d xT[d, t] * w_v[d, f]      (v in [tok, feat] layout,
                                               + fused ones column per head)
  qkT[f, t] = sum_d w_qk[d, f] * xT[d, t]     (q/k in [feat, tok] layout)
  attention per (head h, q-block j of 512, group g of 2 k-tiles):
      S^T[k, q] = sum_d kT[d, k] * qT[d, q]   (only k-tiles <= q-block)
      P^T = exp(S^T / 8)                      (no max-subtraction: scores ~N(0,1))
      causal mask on diagonal groups via gpsimd affine_select (zero where k > q)
      o^T[m, q] = sum_k v_aug[k, m] * P^T[k, q]   (m: 64 v-feats + ones row
                                                   -> row 64 = softmax denominator)
      attn^T[d, q] = o^T[d, q] / o^T[64, q]   (fast recip + bf16 rank-1 PE broadcast
                                               into rows 64.. of the same bank)
  y[t, n] = sum_f attn^T[f, t] * w_o[f, n]

Scheduling: most V/QK projection groups are deferred into a filler queue and
emitted one-per-attention-group between S^T and PV so the PE always has more
queued work than ACT's exp per period -- otherwise the PE idles a few 100ns
every period, HAM re-throttles the clock to 1.2GHz, and every matmul doubles.
The softmax epilogue is similarly split into two stages popped on later
periods (the 1-lane DVE reciprocal takes ~3.4us).
"""

import numpy as np
import ml_dtypes

import concourse.mybir as mybir
import concourse.tile as tile
from concourse import bacc, bass_utils

F32 = mybir.dt.float32
BF16 = mybir.dt.bfloat16

D = 1024          # model dim
T = 2048          # tokens per batch
DH = 64           # head dim
NH_LOC = 8        # heads per core
DT = D // 128     # D tiles (contraction)
TT = T // 128     # token tiles
QB = T // 512     # q blocks of 512
VW = DH + 1       # v width incl ones column


def build_kernel():
    nc = bacc.Bacc()
    xT_d = nc.dram_tensor("xT", [D, T], BF16, kind="ExternalInput")
    wqk_d = nc.dram_tensor("w_qk", [D, 1024], BF16, kind="ExternalInput")
    wv_d = nc.dram_tensor("w_v", [D, 512], BF16, kind="ExternalInput")
    wo_d = nc.dram_tensor("w_o", [512, D], BF16, kind="ExternalInput")
    y_d = nc.dram_tensor("y", [T, D], F32, kind="ExternalOutput")

    with tile.TileContext(nc) as tc:
        with (
            tc.tile_pool(name="big", bufs=1) as big,
            tc.tile_pool(name="ptp", bufs=6) as ptp,
            tc.tile_pool(name="ovp", bufs=2) as ovp,
            tc.tile_pool(name="osb", bufs=1) as osb,
            tc.tile_pool(name="stg", bufs=2) as stg,
            tc.tile_pool(name="ps_st", bufs=2, space="PSUM") as ps_st,
            tc.tile_pool(name="ps_pv", bufs=2, space="PSUM") as ps_pv,
            tc.tile_pool(name="ps_mm", bufs=2, space="PSUM") as ps_mm,
        ):
            xt_a = big.tile([128, DT, T], BF16, tag="xt", name="xt")
            wqk_a = big.tile([128, DT, 1024], BF16, tag="wqk", name="wqk")
            wv_a = big.tile([128, DT, 512], BF16, tag="wv", name="wv")
            wo_a = big.tile([128, 4, 1024], BF16, tag="wo", name="wo")
            qk = [big.tile([128, T], BF16, tag=f"qk{i}", name=f"qk{i}") for i in range(8)]
            attn_t = [big.tile([128, T], BF16, tag=f"attn{i}", name=f"attn{i}") for i in range(4)]
            vsb_t = [big.tile([128, 2, NH_LOC * VW], BF16, tag=f"vsb{i}", name=f"vsb{i}") for i in range(8)]
            ones = big.tile([1, DH], BF16, tag="ones")
            vsb_r = [t.rearrange("p t (h c) -> p t h c", c=VW) for t in vsb_t]

            xT_r = xT_d.rearrange("(i p) t -> p i t", p=128)
            wqk_r = wqk_d.rearrange("(i p) f -> p i f", p=128)
            wv_r = wv_d.rearrange("(i p) f -> p i f", p=128)
            wo_r = wo_d.rearrange("(i p) f -> p i f", p=128)

            # wave 0 split across the three DMA-capable queues: each
            # accumulation step only waits for its own chunk
            rr = [nc.sync, nc.scalar, nc.gpsimd]
            for i in range(DT):
                rr[i % 3].dma_start(wv_a[:, i, :], wv_r[:, i, :])
                rr[(i + 1) % 3].dma_start(xt_a[:, i, 0:512], xT_r[:, i, 0:512])
            for f in (0, 4):  # head-pair 0's q/k weight columns
                nc.scalar.dma_start(
                    wqk_a[:, :, f * 128:(f + 1) * 128],
                    wqk_r[:, :, f * 128:(f + 1) * 128],
                )
            rr2 = [nc.sync, nc.gpsimd]
            for tb in range(1, QB):
                for i in range(DT):
                    rr2[(tb * DT + i) % 2].dma_start(
                        xt_a[:, i, tb * 512:(tb + 1) * 512],
                        xT_r[:, i, tb * 512:(tb + 1) * 512],
                    )
            for f in (1, 5, 2, 6, 3, 7):  # in consumer (head-pair) order
                nc.sync.dma_start(
                    wqk_a[:, :, f * 128:(f + 1) * 128],
                    wqk_r[:, :, f * 128:(f + 1) * 128],
                )
            nc.sync.dma_start(wo_a, wo_r)
            nc.vector.memset(ones, 1.0)
            for t in vsb_r:
                nc.vector.memset(t[:, :, :, DH], 1.0)

            # ---- projection group emitters ----
            def v_group(tt):
                def go():
                    ps = ps_mm.tile([128, 512], F32, tag="mm")
                    for dt in range(DT):
                        nc.tensor.matmul(
                            ps,
                            lhsT=xt_a[:, dt, tt * 128:(tt + 1) * 128],
                            rhs=wv_a[:, dt, :],
                            start=(dt == 0),
                            stop=(dt == DT - 1),
                        )
                    nc.vector.tensor_copy(
                        vsb_r[tt // 2][:, tt % 2, :, 0:DH],
                        ps.rearrange("p (h c) -> p h c", c=DH),
                    )
                return go

            def qk_group(f, tb):
                def go():
                    ps = ps_mm.tile([128, 512], F32, tag="mm")
                    for dt in range(DT):
                        nc.tensor.matmul(
                            ps,
                            lhsT=wqk_a[:, dt, f * 128:(f + 1) * 128],
                            rhs=xt_a[:, dt, tb * 512:(tb + 1) * 512],
                            start=(dt == 0),
                            stop=(dt == DT - 1),
                        )
                    nc.vector.tensor_copy(qk[f][:, tb * 512:(tb + 1) * 512], ps)
                return go

            # up-front: only what attention block (pair0, j=0) needs --
            # V token tiles 0-3 and q/k token-block 0 of head pair 0
            for tt in range(4):
                v_group(tt)()
            qk_group(0, 0)()
            qk_group(4, 0)()

            # the rest becomes PE filler work inside the attention stream;
            # interleaved by deadline (j-block b of pair 0 needs q/k tb<=b and
            # vsb up to tile 4b+3), popped two per period while it lasts
            filler_fast = [
                v_group(4), v_group(5), qk_group(0, 1), qk_group(4, 1),
                v_group(6), v_group(7), v_group(8), v_group(9),
                qk_group(0, 2), qk_group(4, 2), v_group(10), v_group(11),
                v_group(12), v_group(13), qk_group(0, 3), qk_group(4, 3),
                v_group(14), v_group(15),
            ]
            # QK pair p must be projected before head-pair p starts (period
            # 40p); spread the groups across the preceding span so the PE
            # keeps a work surplus the whole way (HAM stays warm)
            filler_slow = []
            for p, t0, step in ((1, 13, 3), (2, 42, 4), (3, 84, 5)):
                for i, tb in enumerate(range(QB)):
                    filler_slow.append((t0 + step * (2 * i), qk_group(p, tb)))
                    filler_slow.append((t0 + step * (2 * i + 1), qk_group(4 + p, tb)))
            filler_slow.sort(key=lambda e: e[0])

            stages = []  # deferred epilogue stages (None = spacer)
            period = {"i": 0}

            def period_extras():
                period["i"] += 1
                if filler_fast:
                    filler_fast.pop(0)()
                    if filler_fast:
                        filler_fast.pop(0)()
                elif filler_slow and period["i"] >= filler_slow[0][0]:
                    filler_slow.pop(0)[1]()
                if stages:
                    s = stages.pop(0)
                    if s is not None:
                        s()

            F16 = mybir.dt.float16
            out_parts = {}

            def out_partA(tt, nb):
                def go():
                    ps = ps_mm.tile([128, 512], F32, tag="mm")
                    for hp4 in range(2):
                        nc.tensor.matmul(
                            ps,
                            lhsT=attn_t[hp4][:, tt * 128:(tt + 1) * 128],
                            rhs=wo_a[:, hp4, nb * 512:(nb + 1) * 512],
                            start=(hp4 == 0),
                            stop=(hp4 == 1),
                        )
                    po = osb.tile([128, 512], F16, tag=f"osb{tt}_{nb}",
                                  name=f"osb{tt}_{nb}")
                    nc.vector.tensor_copy(po, ps)
                    out_parts[(tt, nb)] = po
                return go

            def out_partB(tt, nb):
                def go():
                    ps = ps_mm.tile([128, 512], F32, tag="mm")
                    for hp4 in (2, 3):
                        nc.tensor.matmul(
                            ps,
                            lhsT=attn_t[hp4][:, tt * 128:(tt + 1) * 128],
                            rhs=wo_a[:, hp4, nb * 512:(nb + 1) * 512],
                            start=(hp4 == 2),
                            stop=(hp4 == 3),
                        )
                    ysb = stg.tile([128, 512], F32, tag="y", bufs=4,
                                   name=f"ysb{tt}_{nb}")
                    nc.vector.tensor_add(ysb, out_parts[(tt, nb)], ps)
                    nc.sync.dma_start(
                        y_d[tt * 128:(tt + 1) * 128, nb * 512:(nb + 1) * 512],
                        ysb,
                    )
                return go

            def make_epilogue(hp, j, pvA, pvB):
                """Both heads of the pair at once: denominators into one
                [1,1024] row -> one reciprocal + one bf16 cast; two rank-1
                PE broadcasts into one [128,512] PSUM tile (head B's via
                tile_position col 64); one [128,512] multiply."""
                dn1 = stg.tile([1, 1024], F32, tag="dn", bufs=1, name=f"dn{hp}_{j}")
                rec1 = stg.tile([1, 1024], F32, tag="rec", bufs=1, name=f"rec{hp}_{j}")
                rb1 = stg.tile([1, 1024], BF16, tag="rb", name=f"rb{hp}_{j}")

                def stage1():
                    nc.vector.tensor_copy(dn1[0:1, 0:512], pvA[DH:DH + 1, :])
                    nc.vector.tensor_copy(dn1[0:1, 512:1024], pvB[DH:DH + 1, :])
                    nc.vector.reciprocal_approx_fast(out=rec1, in_=dn1)
                    nc.vector.tensor_copy(rb1, rec1)

                def stage2():
                    ov = ovp.tile([128, 512], F32, tag="ov", name=f"ov{hp}_{j}")
                    nc.vector.tensor_copy(ov[0:DH, :], pvA[0:DH, :])
                    nc.vector.tensor_copy(ov[DH:128, :], pvB[0:DH, :])
                    bc = ps_mm.tile([128, 512], F32, tag="mm")
                    nc.tensor.matmul(bc[0:DH, :], lhsT=ones,
                                     rhs=rb1[0:1, 0:512], start=True, stop=True)
                    nc.tensor.matmul(bc[DH:128, :], lhsT=ones,
                                     rhs=rb1[0:1, 512:1024], start=True,
                                     stop=True, tile_position=(0, 64))
                    nc.vector.tensor_mul(
                        attn_t[hp][:, j * 512:(j + 1) * 512], ov, bc
                    )
                return stage1, stage2

            # ---- attention: head-PAIR outer, j inner, one k-tile per period.
            # The two heads of a pair sit on partitions 0-63 / 64-127 of the
            # same qk tiles, so their K=64 S^T matmuls go to disjoint PE row
            # groups and run concurrently (weight loads overlap too).
            for hp in range(4):
                qTf = qk[hp]
                kTf = qk[4 + hp]
                for j in range(QB):
                    pvA = ps_pv.tile([128, 512], F32, tag="pv")
                    pvB = ps_pv.tile([128, 512], F32, tag="pv")
                    nkt = 4 * (j + 1)
                    pv_queue = []  # PV MMs delayed 2 periods behind S^T/exp
                    for kt in range(nkt):
                        # diagonal k-tiles: q < 128*(kt-4j) is fully masked --
                        # narrow S^T/exp/mask/PV to the live columns
                        q0 = 128 * (kt - 4 * j) if kt >= 4 * j else 0
                        nq = 512 - q0
                        st = ps_st.tile([128, 1024], F32, tag="st")
                        nc.tensor.matmul(
                            st[:, q0:512],
                            lhsT=kTf[0:64, kt * 128:(kt + 1) * 128],
                            rhs=qTf[0:64, j * 512 + q0:(j + 1) * 512],
                            start=True, stop=True,
                        )
                        nc.tensor.matmul(
                            st[:, 512 + q0:1024],
                            lhsT=kTf[64:128, kt * 128:(kt + 1) * 128],
                            rhs=qTf[64:128, j * 512 + q0:(j + 1) * 512],
                            start=True, stop=True,
                        )
                        period_extras()
                        if len(pv_queue) >= 2:
                            pv_queue.pop(0)()
                        pt = ptp.tile([128, 1024], BF16, tag="pt",
                                      name=f"pt{hp}_{j}_{kt}")
                        st_r = st.rearrange("p (h q) -> p h q", h=2)
                        pt_r = pt.rearrange("p (h q) -> p h q", h=2)
                        nc.scalar.activation(
                            pt_r[:, :, q0:512], st_r[:, :, q0:512],
                            mybir.ActivationFunctionType.Exp, scale=0.125
                        )
                        if kt >= 4 * j:  # diagonal k-tile: zero where k > q
                            # in the narrowed frame the condition is just c >= p
                            for half in range(2):
                                nc.gpsimd.affine_select(
                                    out=pt[:, half * 512 + q0:(half + 1) * 512],
                                    in_=pt[:, half * 512 + q0:(half + 1) * 512],
                                    compare_op=mybir.AluOpType.is_ge,
                                    fill=0.0,
                                    base=0,
                                    pattern=[[1, nq]],
                                    channel_multiplier=-1,
                                )

                        def pv_mms(kt=kt, pt=pt, q0=q0):
                            nc.tensor.matmul(
                                pvA[0:VW, q0:512],
                                lhsT=vsb_r[kt // 2][:, kt % 2, 2 * hp, :],
                                rhs=pt[:, q0:512],
                                start=(kt == 0), stop=(kt == nkt - 1),
                            )
                            nc.tensor.matmul(
                                pvB[0:VW, q0:512],
                                lhsT=vsb_r[kt // 2][:, kt % 2, 2 * hp + 1, :],
                                rhs=pt[:, 512 + q0:1024],
                                start=(kt == 0), stop=(kt == nkt - 1),
                            )
                        pv_queue.append(pv_mms)
                    for f_ in pv_queue:
                        f_()
                    if hp == 3 and j == QB - 1:
                        last_epi = make_epilogue(hp, j, pvA, pvB)
                    else:
                        s1, s2 = make_epilogue(hp, j, pvA, pvB)
                        stages.extend([s1, None, s2])
                    if hp == 1:
                        if j < QB - 1:
                            for tt in range(4 * j, 4 * j + 4):
                                for nb in range(2):
                                    stages.append(out_partA(tt, nb))
                        else:
                            partA_j3 = [out_partA(tt, nb)
                                        for tt in range(12, 16)
                                        for nb in range(2)]
                    if hp == 2 and j == 0:
                        stages.extend(partA_j3)
                    if hp == 3 and j < QB - 1:
                        for tt in range(4 * j, 4 * j + 4):
                            for nb in range(2):
                                stages.append(out_partB(tt, nb))

            # tail: kick the final epilogue's DVE chain first, overlap the
            # leftover stage backlog (PE work) with it, then finish the last
            # q-block's partB out-proj
            s1, s2 = last_epi
            s1()
            while stages:
                s = stages.pop(0)
                if s is not None:
                    s()
            s2()
            for tt in range(12, 16):
                for nb in range(2):
                    out_partB(tt, nb)()

    nc.compile()
    return nc


def _shard_inputs(x, w_qkv, w_out):
    """Build the 8 per-core input maps (matmul operands pre-cast to bf16)."""
    bf16 = ml_dtypes.bfloat16
    in_maps = []
    for c in range(8):
        b = c // 2
        hg = c % 2
        q_cols = slice(hg * 512, hg * 512 + 512)
        k_cols = slice(1024 + hg * 512, 1024 + hg * 512 + 512)
        v_cols = slice(2048 + hg * 512, 2048 + hg * 512 + 512)
        in_maps.append({
            "xT": np.ascontiguousarray(x[b].T).astype(bf16),
            "w_qk": np.ascontiguousarray(
                np.concatenate([w_qkv[:, q_cols], w_qkv[:, k_cols]], axis=1)
            ).astype(bf16),
            "w_v": np.ascontiguousarray(w_qkv[:, v_cols]).astype(bf16),
            "w_o": np.ascontiguousarray(w_out[hg * 512:hg * 512 + 512, :]).astype(bf16),
        })
    return in_maps


def _run(inputs, trace=False):
    x = np.asarray(inputs["x"], dtype=np.float32)
    w_qkv = np.asarray(inputs["w_qkv"], dtype=np.float32)
    w_out = np.asarray(inputs["w_out"], dtype=np.float32)
    nc = build_kernel()
    in_maps = _shard_inputs(x, w_qkv, w_out)
    res = None
    for attempt in range(3):
        try:
            res = bass_utils.run_bass_kernel_spmd(
                nc, in_maps, core_ids=list(range(8)), trace=trace
            )
            break
        except Exception:
            if attempt == 2:
                raise
    assert res is not None
    out = np.empty((4, T, D), dtype=np.float32)
    for b in range(4):
        out[b] = res.results[2 * b]["y"] + res.results[2 * b + 1]["y"]
    return out, res


def kernel(**inputs):
    out, _ = _run(inputs, trace=False)
    return out



# revision 18
# speedup vs baseline: 1.1186x; 1.0248x over previous
"""Multi-head causal attention (B=4, T=2048, D=1024, H=16, Dh=64) on 8 trn2 cores.

Sharding: 4-way DP over batch x 2-way TP over heads.
Core c handles batch c//2 and heads (c%2)*8 .. (c%2)*8+7.
Each core computes a partial output [T, D] (its heads' contribution through
w_out rows); host sums the two partials per batch.

Per-core device kernel (bf16 matmul operands, fp32 PSUM accumulation):
  v[t, f]   = sum_d xT[d, t] * w_v[d, f]      (v in [tok, feat] layout,
                                               + fused ones column per head)
  qkT[f, t] = sum_d w_qk[d, f] * xT[d, t]     (q/k in [feat, tok] layout)
  attention per (head h, q-block j of 512, group g of 2 k-tiles):
      S^T[k, q] = sum_d kT[d, k] * qT[d, q]   (only k-tiles <= q-block)
      P^T = exp(S^T / 8)                      (no max-subtraction: scores ~N(0,1))
      causal mask on diagonal groups via gpsimd affine_select (zero where k > q)
      o^T[m, q] = sum_k v_aug[k, m] * P^T[k, q]   (m: 64 v-feats + ones row
                                                   -> row 64 = softmax denominator)
      attn^T[d, q] = o^T[d, q] / o^T[64, q]   (fast recip + bf16 rank-1 PE broadcast
                                               into rows 64.. of the same bank)
  y[t, n] = sum_f attn^T[f, t] * w_o[f, n]

Scheduling: most V/QK projection groups are deferred into a filler queue and
emitted one-per-attention-group between S^T and PV so the PE always has more
queued work than ACT's exp per period -- otherwise the PE idles a few 100ns
every period, HAM re-throttles the clock to 1.2GHz, and every matmul doubles.
The softmax epilogue is similarly split into two stages popped on later
periods (the 1-lane DVE reciprocal takes ~3.4us).
"""

import numpy as np
import ml_dtypes

import concourse.mybir as mybir
import concourse.tile as tile
from concourse import bacc, bass_utils

F32 = mybir.dt.float32
BF16 = mybir.dt.bfloat16

D = 1024          # model dim
T = 2048          # tokens per batch
DH = 64           # head dim
NH_LOC = 8        # heads per core
DT = D // 128     # D tiles (contraction)
TT = T // 128     # token tiles
QB = T // 512     # q blocks of 512
VW = DH + 1       # v width incl ones column


def build_kernel():
    nc = bacc.Bacc()
    xT_d = nc.dram_tensor("xT", [D, T], BF16, kind="ExternalInput")
    wqk_d = nc.dram_tensor("w_qk", [D, 1024], BF16, kind="ExternalInput")
    wv_d = nc.dram_tensor("w_v", [D, 512], BF16, kind="ExternalInput")
    wo_d = nc.dram_tensor("w_o", [512, D], BF16, kind="ExternalInput")
    y_d = nc.dram_tensor("y", [T, D], F32, kind="ExternalOutput")

    with tile.TileContext(nc) as tc:
        with (
            tc.tile_pool(name="big", bufs=1) as big,
            tc.tile_pool(name="ptp", bufs=6) as ptp,
            tc.tile_pool(name="ovp", bufs=3) as ovp,
            tc.tile_pool(name="osb", bufs=1) as osb,
            tc.tile_pool(name="stg", bufs=2) as stg,
            tc.tile_pool(name="ps_st", bufs=2, space="PSUM") as ps_st,
            tc.tile_pool(name="ps_pv", bufs=2, space="PSUM") as ps_pv,
            tc.tile_pool(name="ps_mm", bufs=2, space="PSUM") as ps_mm,
        ):
            xt_a = big.tile([128, DT, T], BF16, tag="xt", name="xt")
            wqk_a = big.tile([128, DT, 1024], BF16, tag="wqk", name="wqk")
            wv_a = big.tile([128, DT, 512], BF16, tag="wv", name="wv")
            wo_a = big.tile([128, 4, 1024], BF16, tag="wo", name="wo")
            qk = [big.tile([128, T], BF16, tag=f"qk{i}", name=f"qk{i}") for i in range(8)]
            attn_t = [big.tile([128, T], BF16, tag=f"attn{i}", name=f"attn{i}") for i in range(4)]
            vsb_t = [big.tile([128, 2, NH_LOC * VW], BF16, tag=f"vsb{i}", name=f"vsb{i}") for i in range(8)]
            ones = big.tile([1, DH], BF16, tag="ones")
            vsb_r = [t.rearrange("p t (h c) -> p t h c", c=VW) for t in vsb_t]

            xT_r = xT_d.rearrange("(i p) t -> p i t", p=128)
            wqk_r = wqk_d.rearrange("(i p) f -> p i f", p=128)
            wv_r = wv_d.rearrange("(i p) f -> p i f", p=128)
            wo_r = wo_d.rearrange("(i p) f -> p i f", p=128)

            # wave 0 split across the three DMA-capable queues: each
            # accumulation step only waits for its own chunk
            rr = [nc.sync, nc.scalar, nc.gpsimd]
            for i in range(DT):
                rr[i % 3].dma_start(wv_a[:, i, :], wv_r[:, i, :])
                rr[(i + 1) % 3].dma_start(xt_a[:, i, 0:512], xT_r[:, i, 0:512])
            for f in (0, 4):  # head-pair 0's q/k weight columns
                nc.scalar.dma_start(
                    wqk_a[:, :, f * 128:(f + 1) * 128],
                    wqk_r[:, :, f * 128:(f + 1) * 128],
                )
            rr2 = [nc.sync, nc.gpsimd]
            for tb in range(1, QB):
                for i in range(DT):
                    rr2[(tb * DT + i) % 2].dma_start(
                        xt_a[:, i, tb * 512:(tb + 1) * 512],
                        xT_r[:, i, tb * 512:(tb + 1) * 512],
                    )
            for f in (1, 5, 2, 6, 3, 7):  # in consumer (head-pair) order
                nc.sync.dma_start(
                    wqk_a[:, :, f * 128:(f + 1) * 128],
                    wqk_r[:, :, f * 128:(f + 1) * 128],
                )
            nc.sync.dma_start(wo_a, wo_r)
            nc.vector.memset(ones, 1.0)
            for t in vsb_r:
                nc.vector.memset(t[:, :, :, DH], 1.0)

            # ---- projection group emitters ----
            def v_group(tt):
                def go():
                    ps = ps_mm.tile([128, 512], F32, tag="mm")
                    for dt in range(DT):
                        nc.tensor.matmul(
                            ps,
                            lhsT=xt_a[:, dt, tt * 128:(tt + 1) * 128],
                            rhs=wv_a[:, dt, :],
                            start=(dt == 0),
                            stop=(dt == DT - 1),
                        )
                    nc.vector.tensor_copy(
                        vsb_r[tt // 2][:, tt % 2, :, 0:DH],
                        ps.rearrange("p (h c) -> p h c", c=DH),
                    )
                return go

            def qk_group(f, tb):
                def go():
                    ps = ps_mm.tile([128, 512], F32, tag="mm")
                    for dt in range(DT):
                        nc.tensor.matmul(
                            ps,
                            lhsT=wqk_a[:, dt, f * 128:(f + 1) * 128],
                            rhs=xt_a[:, dt, tb * 512:(tb + 1) * 512],
                            start=(dt == 0),
                            stop=(dt == DT - 1),
                        )
                    nc.vector.tensor_copy(qk[f][:, tb * 512:(tb + 1) * 512], ps)
                return go

            # up-front: only what attention block (pair0, j=0) needs --
            # V token tiles 0-3 and q/k token-block 0 of head pair 0
            for tt in range(4):
                v_group(tt)()
            qk_group(0, 0)()
            qk_group(4, 0)()

            # the rest becomes PE filler work inside the attention stream;
            # interleaved by deadline (j-block b of pair 0 needs q/k tb<=b and
            # vsb up to tile 4b+3), popped two per period while it lasts
            filler_fast = [
                v_group(4), v_group(5), qk_group(0, 1), qk_group(4, 1),
                v_group(6), v_group(7), v_group(8), v_group(9),
                qk_group(0, 2), qk_group(4, 2), v_group(10), v_group(11),
                v_group(12), v_group(13), qk_group(0, 3), qk_group(4, 3),
                v_group(14), v_group(15),
            ]
            # QK pair p must be projected before head-pair p starts (period
            # 40p); spread the groups across the preceding span so the PE
            # keeps a work surplus the whole way (HAM stays warm)
            filler_slow = []
            for p, t0, step in ((1, 13, 3), (2, 42, 4), (3, 84, 5)):
                for i, tb in enumerate(range(QB)):
                    filler_slow.append((t0 + step * (2 * i), qk_group(p, tb)))
                    filler_slow.append((t0 + step * (2 * i + 1), qk_group(4 + p, tb)))
            filler_slow.sort(key=lambda e: e[0])

            stages = []  # deferred epilogue stages (None = spacer)
            period = {"i": 0}

            def period_extras():
                period["i"] += 1
                if filler_fast:
                    filler_fast.pop(0)()
                    if filler_fast:
                        filler_fast.pop(0)()
                elif filler_slow and period["i"] >= filler_slow[0][0]:
                    filler_slow.pop(0)[1]()
                if stages:
                    s = stages.pop(0)
                    if s is not None:
                        s()

            F16 = mybir.dt.float16
            out_parts = {}

            def out_partA(tt, nb):
                def go():
                    ps = ps_mm.tile([128, 512], F32, tag="mm")
                    for hp4 in range(2):
                        nc.tensor.matmul(
                            ps,
                            lhsT=attn_t[hp4][:, tt * 128:(tt + 1) * 128],
                            rhs=wo_a[:, hp4, nb * 512:(nb + 1) * 512],
                            start=(hp4 == 0),
                            stop=(hp4 == 1),
                        )
                    po = osb.tile([128, 512], F16, tag=f"osb{tt}_{nb}",
                                  name=f"osb{tt}_{nb}")
                    nc.vector.tensor_copy(po, ps)
                    out_parts[(tt, nb)] = po
                return go

            def out_partB(tt, nb):
                def go():
                    ps = ps_mm.tile([128, 512], F32, tag="mm")
                    for hp4 in (2, 3):
                        nc.tensor.matmul(
                            ps,
                            lhsT=attn_t[hp4][:, tt * 128:(tt + 1) * 128],
                            rhs=wo_a[:, hp4, nb * 512:(nb + 1) * 512],
                            start=(hp4 == 2),
                            stop=(hp4 == 3),
                        )
                    ysb = stg.tile([128, 512], F32, tag="y", bufs=4,
                                   name=f"ysb{tt}_{nb}")
                    nc.vector.tensor_add(ysb, out_parts[(tt, nb)], ps)
                    nc.sync.dma_start(
                        y_d[tt * 128:(tt + 1) * 128, nb * 512:(nb + 1) * 512],
                        ysb,
                    )
                return go

            def make_epilogue(hp, j, pvA, pvB):
                """Both heads of the pair at once: denominators into one
                [1,1024] row -> one reciprocal + one bf16 cast; two rank-1
                PE broadcasts into one [128,512] PSUM tile (head B's via
                tile_position col 64); one [128,512] multiply."""
                dn1 = stg.tile([1, 1024], F32, tag="dn", bufs=1, name=f"dn{hp}_{j}")
                rec1 = stg.tile([1, 1024], F32, tag="rec", bufs=1, name=f"rec{hp}_{j}")
                rb1 = stg.tile([1, 1024], BF16, tag="rb", name=f"rb{hp}_{j}")

                def stage1():
                    nc.vector.tensor_copy(dn1[0:1, 0:512], pvA[DH:DH + 1, :])
                    nc.vector.tensor_copy(dn1[0:1, 512:1024], pvB[DH:DH + 1, :])
                    nc.vector.reciprocal_approx_fast(out=rec1, in_=dn1)
                    nc.vector.tensor_copy(rb1, rec1)

                # evacuate the PV accumulator banks immediately so the
                # next q-block's first PV matmul never stalls on the WAR
                ov = ovp.tile([128, 512], F32, tag="ov", name=f"ov{hp}_{j}")
                nc.vector.tensor_copy(ov[0:DH, :], pvA[0:DH, :])
                nc.vector.tensor_copy(ov[DH:128, :], pvB[0:DH, :])

                def stage2():
                    bc = ps_mm.tile([128, 512], F32, tag="mm")
                    nc.tensor.matmul(bc[0:DH, :], lhsT=ones,
                                     rhs=rb1[0:1, 0:512], start=True, stop=True)
                    nc.tensor.matmul(bc[DH:128, :], lhsT=ones,
                                     rhs=rb1[0:1, 512:1024], start=True,
                                     stop=True, tile_position=(0, 64))
                    nc.vector.tensor_mul(
                        attn_t[hp][:, j * 512:(j + 1) * 512], ov, bc
                    )
                return stage1, stage2

            # ---- attention: head-PAIR outer, j inner, one k-tile per period.
            # The two heads of a pair sit on partitions 0-63 / 64-127 of the
            # same qk tiles, so their K=64 S^T matmuls go to disjoint PE row
            # groups and run concurrently (weight loads overlap too).
            for hp in range(4):
                qTf = qk[hp]
                kTf = qk[4 + hp]
                for j in range(QB):
                    pvA = ps_pv.tile([128, 512], F32, tag="pv")
                    pvB = ps_pv.tile([128, 512], F32, tag="pv")
                    nkt = 4 * (j + 1)
                    pv_queue = []  # PV MMs delayed 2 periods behind S^T/exp
                    for kt in range(nkt):
                        # diagonal k-tiles: q < 128*(kt-4j) is fully masked --
                        # narrow S^T/exp/mask/PV to the live columns
                        q0 = 128 * (kt - 4 * j) if kt >= 4 * j else 0
                        nq = 512 - q0
                        st = ps_st.tile([128, 1024], F32, tag="st")
                        nc.tensor.matmul(
                            st[:, q0:512],
                            lhsT=kTf[0:64, kt * 128:(kt + 1) * 128],
                            rhs=qTf[0:64, j * 512 + q0:(j + 1) * 512],
                            start=True, stop=True,
                        )
                        nc.tensor.matmul(
                            st[:, 512 + q0:1024],
                            lhsT=kTf[64:128, kt * 128:(kt + 1) * 128],
                            rhs=qTf[64:128, j * 512 + q0:(j + 1) * 512],
                            start=True, stop=True,
                        )
                        period_extras()
                        if len(pv_queue) >= 2:
                            pv_queue.pop(0)()
                        pt = ptp.tile([128, 1024], BF16, tag="pt",
                                      name=f"pt{hp}_{j}_{kt}")
                        st_r = st.rearrange("p (h q) -> p h q", h=2)
                        pt_r = pt.rearrange("p (h q) -> p h q", h=2)
                        nc.scalar.activation(
                            pt_r[:, :, q0:512], st_r[:, :, q0:512],
                            mybir.ActivationFunctionType.Exp, scale=0.125
                        )
                        if kt >= 4 * j:  # diagonal k-tile: zero where k > q
                            # in the narrowed frame the condition is just c >= p
                            for half in range(2):
                                nc.gpsimd.affine_select(
                                    out=pt[:, half * 512 + q0:(half + 1) * 512],
                                    in_=pt[:, half * 512 + q0:(half + 1) * 512],
                                    compare_op=mybir.AluOpType.is_ge,
                                    fill=0.0,
                                    base=0,
                                    pattern=[[1, nq]],
                                    channel_multiplier=-1,
                                )

                        def pv_mms(kt=kt, pt=pt, q0=q0):
                            nc.tensor.matmul(
                                pvA[0:VW, q0:512],
                                lhsT=vsb_r[kt // 2][:, kt % 2, 2 * hp, :],
                                rhs=pt[:, q0:512],
                                start=(kt == 0), stop=(kt == nkt - 1),
                            )
                            nc.tensor.matmul(
                                pvB[0:VW, q0:512],
                                lhsT=vsb_r[kt // 2][:, kt % 2, 2 * hp + 1, :],
                                rhs=pt[:, 512 + q0:1024],
                                start=(kt == 0), stop=(kt == nkt - 1),
                            )
                        pv_queue.append(pv_mms)
                    for f_ in pv_queue:
                        f_()
                    if hp == 3 and j == QB - 1:
                        last_epi = make_epilogue(hp, j, pvA, pvB)
                    else:
                        s1, s2 = make_epilogue(hp, j, pvA, pvB)
                        stages.extend([s1, None, s2])
                    if hp == 1:
                        if j < QB - 1:
                            for tt in range(4 * j, 4 * j + 4):
                                for nb in range(2):
                                    stages.append(out_partA(tt, nb))
                        else:
                            partA_j3 = [out_partA(tt, nb)
                                        for tt in range(12, 16)
                                        for nb in range(2)]
                    if hp == 2 and j == 0:
                        stages.extend(partA_j3)
                    if hp == 3 and j < QB - 1:
                        for tt in range(4 * j, 4 * j + 4):
                            for nb in range(2):
                                stages.append(out_partB(tt, nb))

            # tail: kick the final epilogue's DVE chain first, overlap the
            # leftover stage backlog (PE work) with it, then finish the last
            # q-block's partB out-proj
            s1, s2 = last_epi
            s1()
            while stages:
                s = stages.pop(0)
                if s is not None:
                    s()
            s2()
            for tt in range(12, 16):
                for nb in range(2):
                    out_partB(tt, nb)()

    nc.compile()
    return nc


def _shard_inputs(x, w_qkv, w_out):
    """Build the 8 per-core input maps (matmul operands pre-cast to bf16)."""
    bf16 = ml_dtypes.bfloat16
    in_maps = []
    for c in range(8):
        b = c // 2
        hg = c % 2
        q_cols = slice(hg * 512, hg * 512 + 512)
        k_cols = slice(1024 + hg * 512, 1024 + hg * 512 + 512)
        v_cols = slice(2048 + hg * 512, 2048 + hg * 512 + 512)
        in_maps.append({
            "xT": np.ascontiguousarray(x[b].T).astype(bf16),
            "w_qk": np.ascontiguousarray(
                np.concatenate([w_qkv[:, q_cols], w_qkv[:, k_cols]], axis=1)
            ).astype(bf16),
            "w_v": np.ascontiguousarray(w_qkv[:, v_cols]).astype(bf16),
            "w_o": np.ascontiguousarray(w_out[hg * 512:hg * 512 + 512, :]).astype(bf16),
        })
    return in_maps


def _run(inputs, trace=False):
    x = np.asarray(inputs["x"], dtype=np.float32)
    w_qkv = np.asarray(inputs["w_qkv"], dtype=np.float32)
    w_out = np.asarray(inputs["w_out"], dtype=np.float32)
    nc = build_kernel()
    in_maps = _shard_inputs(x, w_qkv, w_out)
    res = None
    for attempt in range(3):
        try:
            res = bass_utils.run_bass_kernel_spmd(
                nc, in_maps, core_ids=list(range(8)), trace=trace
            )
            break
        except Exception:
            if attempt == 2:
                raise
    assert res is not None
    out = np.empty((4, T, D), dtype=np.float32)
    for b in range(4):
        out[b] = res.results[2 * b]["y"] + res.results[2 * b + 1]["y"]
    return out, res


def kernel(**inputs):
    out, _ = _run(inputs, trace=False)
    return out



# revision 19
# speedup vs baseline: 1.1290x; 1.0093x over previous
"""Multi-head causal attention (B=4, T=2048, D=1024, H=16, Dh=64) on 8 trn2 cores.

Sharding: 4-way DP over batch x 2-way TP over heads.
Core c handles batch c//2 and heads (c%2)*8 .. (c%2)*8+7.
Each core computes a partial output [T, D] (its heads' contribution through
w_out rows); host sums the two partials per batch.

Per-core device kernel (bf16 matmul operands, fp32 PSUM accumulation):
  v[t, f]   = sum_d xT[d, t] * w_v[d, f]      (v in [tok, feat] layout,
                                               + fused ones column per head)
  qkT[f, t] = sum_d w_qk[d, f] * xT[d, t]     (q/k in [feat, tok] layout)
  attention per (head h, q-block j of 512, group g of 2 k-tiles):
      S^T[k, q] = sum_d kT[d, k] * qT[d, q]   (only k-tiles <= q-block)
      P^T = exp(S^T / 8)                      (no max-subtraction: scores ~N(0,1))
      causal mask on diagonal groups via gpsimd affine_select (zero where k > q)
      o^T[m, q] = sum_k v_aug[k, m] * P^T[k, q]   (m: 64 v-feats + ones row
                                                   -> row 64 = softmax denominator)
      attn^T[d, q] = o^T[d, q] / o^T[64, q]   (fast recip + bf16 rank-1 PE broadcast
                                               into rows 64.. of the same bank)
  y[t, n] = sum_f attn^T[f, t] * w_o[f, n]

Scheduling: most V/QK projection groups are deferred into a filler queue and
emitted one-per-attention-group between S^T and PV so the PE always has more
queued work than ACT's exp per period -- otherwise the PE idles a few 100ns
every period, HAM re-throttles the clock to 1.2GHz, and every matmul doubles.
The softmax epilogue is similarly split into two stages popped on later
periods (the 1-lane DVE reciprocal takes ~3.4us).
"""

import numpy as np
import ml_dtypes

import concourse.mybir as mybir
import concourse.tile as tile
from concourse import bacc, bass_utils

F32 = mybir.dt.float32
BF16 = mybir.dt.bfloat16

D = 1024          # model dim
T = 2048          # tokens per batch
DH = 64           # head dim
NH_LOC = 8        # heads per core
DT = D // 128     # D tiles (contraction)
TT = T // 128     # token tiles
QB = T // 512     # q blocks of 512
VW = DH + 1       # v width incl ones column


def build_kernel():
    nc = bacc.Bacc()
    xT_d = nc.dram_tensor("xT", [D, T], BF16, kind="ExternalInput")
    wqk_d = nc.dram_tensor("w_qk", [D, 1024], BF16, kind="ExternalInput")
    wv_d = nc.dram_tensor("w_v", [D, 512], BF16, kind="ExternalInput")
    wo_d = nc.dram_tensor("w_o", [512, D], BF16, kind="ExternalInput")
    y_d = nc.dram_tensor("y", [T, D], F32, kind="ExternalOutput")

    with tile.TileContext(nc) as tc:
        with (
            tc.tile_pool(name="big", bufs=1) as big,
            tc.tile_pool(name="ptp", bufs=6) as ptp,
            tc.tile_pool(name="ovp", bufs=3) as ovp,
            tc.tile_pool(name="osb", bufs=1) as osb,
            tc.tile_pool(name="stg", bufs=2) as stg,
            tc.tile_pool(name="ps_st", bufs=2, space="PSUM") as ps_st,
            tc.tile_pool(name="ps_pv", bufs=2, space="PSUM") as ps_pv,
            tc.tile_pool(name="ps_mm", bufs=2, space="PSUM") as ps_mm,
        ):
            xt_a = big.tile([128, DT, T], BF16, tag="xt", name="xt")
            wqk_a = big.tile([128, DT, 1024], BF16, tag="wqk", name="wqk")
            wv_a = big.tile([128, DT, 512], BF16, tag="wv", name="wv")
            wo_a = big.tile([128, 4, 1024], BF16, tag="wo", name="wo")
            qk = [big.tile([128, T], BF16, tag=f"qk{i}", name=f"qk{i}") for i in range(8)]
            attn_t = [big.tile([128, T], BF16, tag=f"attn{i}", name=f"attn{i}") for i in range(4)]
            vsb_t = [big.tile([128, 2, NH_LOC * VW], BF16, tag=f"vsb{i}", name=f"vsb{i}") for i in range(8)]
            ones = big.tile([1, DH], BF16, tag="ones")
            vsb_r = [t.rearrange("p t (h c) -> p t h c", c=VW) for t in vsb_t]

            xT_r = xT_d.rearrange("(i p) t -> p i t", p=128)
            wqk_r = wqk_d.rearrange("(i p) f -> p i f", p=128)
            wv_r = wv_d.rearrange("(i p) f -> p i f", p=128)
            wo_r = wo_d.rearrange("(i p) f -> p i f", p=128)

            # wave 0 split across the three DMA-capable queues: each
            # accumulation step only waits for its own chunk
            rr = [nc.sync, nc.scalar, nc.gpsimd]
            for i in range(DT):
                rr[i % 3].dma_start(wv_a[:, i, :], wv_r[:, i, :])
                rr[(i + 1) % 3].dma_start(xt_a[:, i, 0:512], xT_r[:, i, 0:512])
            for f in (0, 4):  # head-pair 0's q/k weight columns
                nc.scalar.dma_start(
                    wqk_a[:, :, f * 128:(f + 1) * 128],
                    wqk_r[:, :, f * 128:(f + 1) * 128],
                )
            rr2 = [nc.sync, nc.gpsimd]
            for tb in range(1, QB):
                for i in range(DT):
                    rr2[(tb * DT + i) % 2].dma_start(
                        xt_a[:, i, tb * 512:(tb + 1) * 512],
                        xT_r[:, i, tb * 512:(tb + 1) * 512],
                    )
            for f in (1, 5, 2, 6, 3, 7):  # in consumer (head-pair) order
                nc.sync.dma_start(
                    wqk_a[:, :, f * 128:(f + 1) * 128],
                    wqk_r[:, :, f * 128:(f + 1) * 128],
                )
            nc.sync.dma_start(wo_a, wo_r)
            nc.vector.memset(ones, 1.0)
            for t in vsb_r:
                nc.vector.memset(t[:, :, :, DH], 1.0)

            # ---- projection group emitters ----
            def v_group(tt):
                def go():
                    ps = ps_mm.tile([128, 512], F32, tag="mm")
                    for dt in range(DT):
                        nc.tensor.matmul(
                            ps,
                            lhsT=xt_a[:, dt, tt * 128:(tt + 1) * 128],
                            rhs=wv_a[:, dt, :],
                            start=(dt == 0),
                            stop=(dt == DT - 1),
                        )
                    nc.vector.tensor_copy(
                        vsb_r[tt // 2][:, tt % 2, :, 0:DH],
                        ps.rearrange("p (h c) -> p h c", c=DH),
                    )
                return go

            def qk_group(f, tb):
                def go():
                    ps = ps_mm.tile([128, 512], F32, tag="mm")
                    for dt in range(DT):
                        nc.tensor.matmul(
                            ps,
                            lhsT=wqk_a[:, dt, f * 128:(f + 1) * 128],
                            rhs=xt_a[:, dt, tb * 512:(tb + 1) * 512],
                            start=(dt == 0),
                            stop=(dt == DT - 1),
                        )
                    nc.vector.tensor_copy(qk[f][:, tb * 512:(tb + 1) * 512], ps)
                return go

            # up-front: only what attention block (pair0, j=0) needs --
            # V token tiles 0-3 and q/k token-block 0 of head pair 0
            for tt in range(4):
                v_group(tt)()
            qk_group(0, 0)()
            qk_group(4, 0)()

            # the rest becomes PE filler work inside the attention stream;
            # interleaved by deadline (j-block b of pair 0 needs q/k tb<=b and
            # vsb up to tile 4b+3), popped two per period while it lasts
            filler_fast = [
                v_group(4), v_group(5), qk_group(0, 1), qk_group(4, 1),
                v_group(6), v_group(7), v_group(8), v_group(9),
                qk_group(0, 2), qk_group(4, 2), v_group(10), v_group(11),
                v_group(12), v_group(13), qk_group(0, 3), qk_group(4, 3),
                v_group(14), v_group(15),
            ]
            # QK pair p must be projected before head-pair p starts (period
            # 40p); spread the groups across the preceding span so the PE
            # keeps a work surplus the whole way (HAM stays warm)
            filler_slow = []
            for p, t0, step in ((1, 13, 3), (2, 42, 4), (3, 84, 5)):
                for i, tb in enumerate(range(QB)):
                    filler_slow.append((t0 + step * (2 * i), qk_group(p, tb)))
                    filler_slow.append((t0 + step * (2 * i + 1), qk_group(4 + p, tb)))
            filler_slow.sort(key=lambda e: e[0])

            stages = []  # deferred epilogue stages (None = spacer)
            period = {"i": 0}

            def period_extras():
                period["i"] += 1
                if filler_fast:
                    filler_fast.pop(0)()
                    if filler_fast:
                        filler_fast.pop(0)()
                elif filler_slow and period["i"] >= filler_slow[0][0]:
                    filler_slow.pop(0)[1]()
                if stages:
                    s = stages.pop(0)
                    if s is not None:
                        s()

            F16 = mybir.dt.float16
            out_parts = {}

            def out_partA(tt, nb):
                def go():
                    ps = ps_mm.tile([128, 512], F32, tag="mm")
                    for hp4 in range(2):
                        nc.tensor.matmul(
                            ps,
                            lhsT=attn_t[hp4][:, tt * 128:(tt + 1) * 128],
                            rhs=wo_a[:, hp4, nb * 512:(nb + 1) * 512],
                            start=(hp4 == 0),
                            stop=(hp4 == 1),
                        )
                    po = osb.tile([128, 512], F16, tag=f"osb{tt}_{nb}",
                                  name=f"osb{tt}_{nb}")
                    nc.vector.tensor_copy(po, ps)
                    out_parts[(tt, nb)] = po
                return go

            def out_partB(tt, nb):
                def go():
                    ps = ps_mm.tile([128, 512], F32, tag="mm")
                    for hp4 in (2, 3):
                        nc.tensor.matmul(
                            ps,
                            lhsT=attn_t[hp4][:, tt * 128:(tt + 1) * 128],
                            rhs=wo_a[:, hp4, nb * 512:(nb + 1) * 512],
                            start=(hp4 == 2),
                            stop=(hp4 == 3),
                        )
                    ysb = stg.tile([128, 512], F32, tag="y", bufs=4,
                                   name=f"ysb{tt}_{nb}")
                    nc.vector.tensor_add(ysb, out_parts[(tt, nb)], ps)
                    nc.sync.dma_start(
                        y_d[tt * 128:(tt + 1) * 128, nb * 512:(nb + 1) * 512],
                        ysb,
                    )
                return go

            def make_epilogue(hp, j, pvA, pvB):
                """Both heads of the pair at once: denominators into one
                [1,1024] row -> one reciprocal + one bf16 cast; two rank-1
                PE broadcasts into one [128,512] PSUM tile (head B's via
                tile_position col 64); one [128,512] multiply."""
                dn1 = stg.tile([1, 1024], F32, tag="dn", bufs=2, name=f"dn{hp}_{j}")
                rec1 = stg.tile([1, 1024], F32, tag="rec", bufs=1, name=f"rec{hp}_{j}")
                rb1 = stg.tile([1, 1024], BF16, tag="rb", name=f"rb{hp}_{j}")

                # evacuate the PV accumulator banks (incl. denominator
                # rows) immediately so the next q-block's first PV matmul
                # never stalls on the WAR
                ov = ovp.tile([128, 512], F32, tag="ov", name=f"ov{hp}_{j}")
                nc.vector.tensor_copy(dn1[0:1, 0:512], pvA[DH:DH + 1, :])
                nc.vector.tensor_copy(dn1[0:1, 512:1024], pvB[DH:DH + 1, :])
                nc.vector.tensor_copy(ov[0:DH, :], pvA[0:DH, :])
                nc.vector.tensor_copy(ov[DH:128, :], pvB[0:DH, :])

                def stage1():
                    nc.vector.reciprocal_approx_fast(out=rec1, in_=dn1)
                    nc.vector.tensor_copy(rb1, rec1)

                def stage2():
                    bc = ps_mm.tile([128, 512], F32, tag="mm")
                    nc.tensor.matmul(bc[0:DH, :], lhsT=ones,
                                     rhs=rb1[0:1, 0:512], start=True, stop=True)
                    nc.tensor.matmul(bc[DH:128, :], lhsT=ones,
                                     rhs=rb1[0:1, 512:1024], start=True,
                                     stop=True, tile_position=(0, 64))
                    nc.vector.tensor_mul(
                        attn_t[hp][:, j * 512:(j + 1) * 512], ov, bc
                    )
                return stage1, stage2

            # ---- attention: head-PAIR outer, j inner, one k-tile per period.
            # The two heads of a pair sit on partitions 0-63 / 64-127 of the
            # same qk tiles, so their K=64 S^T matmuls go to disjoint PE row
            # groups and run concurrently (weight loads overlap too).
            for hp in range(4):
                qTf = qk[hp]
                kTf = qk[4 + hp]
                for j in range(QB):
                    pvA = ps_pv.tile([128, 512], F32, tag="pv")
                    pvB = ps_pv.tile([128, 512], F32, tag="pv")
                    nkt = 4 * (j + 1)
                    pv_queue = []  # PV MMs delayed 2 periods behind S^T/exp
                    for kt in range(nkt):
                        # diagonal k-tiles: q < 128*(kt-4j) is fully masked --
                        # narrow S^T/exp/mask/PV to the live columns
                        q0 = 128 * (kt - 4 * j) if kt >= 4 * j else 0
                        nq = 512 - q0
                        st = ps_st.tile([128, 1024], F32, tag="st")
                        nc.tensor.matmul(
                            st[:, q0:512],
                            lhsT=kTf[0:64, kt * 128:(kt + 1) * 128],
                            rhs=qTf[0:64, j * 512 + q0:(j + 1) * 512],
                            start=True, stop=True,
                        )
                        nc.tensor.matmul(
                            st[:, 512 + q0:1024],
                            lhsT=kTf[64:128, kt * 128:(kt + 1) * 128],
                            rhs=qTf[64:128, j * 512 + q0:(j + 1) * 512],
                            start=True, stop=True,
                        )
                        period_extras()
                        if len(pv_queue) >= 2:
                            pv_queue.pop(0)()
                        pt = ptp.tile([128, 1024], BF16, tag="pt",
                                      name=f"pt{hp}_{j}_{kt}")
                        st_r = st.rearrange("p (h q) -> p h q", h=2)
                        pt_r = pt.rearrange("p (h q) -> p h q", h=2)
                        nc.scalar.activation(
                            pt_r[:, :, q0:512], st_r[:, :, q0:512],
                            mybir.ActivationFunctionType.Exp, scale=0.125
                        )
                        if kt >= 4 * j:  # diagonal k-tile: zero where k > q
                            # in the narrowed frame the condition is just c >= p
                            for half in range(2):
                                nc.gpsimd.affine_select(
                                    out=pt[:, half * 512 + q0:(half + 1) * 512],
                                    in_=pt[:, half * 512 + q0:(half + 1) * 512],
                                    compare_op=mybir.AluOpType.is_ge,
                                    fill=0.0,
                                    base=0,
                                    pattern=[[1, nq]],
                                    channel_multiplier=-1,
                                )

                        def pv_mms(kt=kt, pt=pt, q0=q0):
                            nc.tensor.matmul(
                                pvA[0:VW, q0:512],
                                lhsT=vsb_r[kt // 2][:, kt % 2, 2 * hp, :],
                                rhs=pt[:, q0:512],
                                start=(kt == 0), stop=(kt == nkt - 1),
                            )
                            nc.tensor.matmul(
                                pvB[0:VW, q0:512],
                                lhsT=vsb_r[kt // 2][:, kt % 2, 2 * hp + 1, :],
                                rhs=pt[:, 512 + q0:1024],
                                start=(kt == 0), stop=(kt == nkt - 1),
                            )
                        pv_queue.append(pv_mms)
                    for f_ in pv_queue:
                        f_()
                    if hp == 3 and j == QB - 1:
                        last_epi = make_epilogue(hp, j, pvA, pvB)
                    else:
                        s1, s2 = make_epilogue(hp, j, pvA, pvB)
                        stages.extend([s1, None, s2])
                    if hp == 1:
                        if j < QB - 1:
                            for tt in range(4 * j, 4 * j + 4):
                                for nb in range(2):
                                    stages.append(out_partA(tt, nb))
                        else:
                            partA_j3 = [out_partA(tt, nb)
                                        for tt in range(12, 16)
                                        for nb in range(2)]
                    if hp == 2 and j == 0:
                        stages.extend(partA_j3)
                    if hp == 3 and j < QB - 1:
                        for tt in range(4 * j, 4 * j + 4):
                            for nb in range(2):
                                stages.append(out_partB(tt, nb))

            # tail: kick the final epilogue's DVE chain first, overlap the
            # leftover stage backlog (PE work) with it, then finish the last
            # q-block's partB out-proj
            s1, s2 = last_epi
            s1()
            while stages:
                s = stages.pop(0)
                if s is not None:
                    s()
            s2()
            for tt in range(12, 16):
                for nb in range(2):
                    out_partB(tt, nb)()

    nc.compile()
    return nc


def _shard_inputs(x, w_qkv, w_out):
    """Build the 8 per-core input maps (matmul operands pre-cast to bf16)."""
    bf16 = ml_dtypes.bfloat16
    in_maps = []
    for c in range(8):
        b = c // 2
        hg = c % 2
        q_cols = slice(hg * 512, hg * 512 + 512)
        k_cols = slice(1024 + hg * 512, 1024 + hg * 512 + 512)
        v_cols = slice(2048 + hg * 512, 2048 + hg * 512 + 512)
        in_maps.append({
            "xT": np.ascontiguousarray(x[b].T).astype(bf16),
            "w_qk": np.ascontiguousarray(
                np.concatenate([w_qkv[:, q_cols], w_qkv[:, k_cols]], axis=1)
            ).astype(bf16),
            "w_v": np.ascontiguousarray(w_qkv[:, v_cols]).astype(bf16),
            "w_o": np.ascontiguousarray(w_out[hg * 512:hg * 512 + 512, :]).astype(bf16),
        })
    return in_maps


def _run(inputs, trace=False):
    x = np.asarray(inputs["x"], dtype=np.float32)
    w_qkv = np.asarray(inputs["w_qkv"], dtype=np.float32)
    w_out = np.asarray(inputs["w_out"], dtype=np.float32)
    nc = build_kernel()
    in_maps = _shard_inputs(x, w_qkv, w_out)
    res = None
    for attempt in range(3):
        try:
            res = bass_utils.run_bass_kernel_spmd(
                nc, in_maps, core_ids=list(range(8)), trace=trace
            )
            break
        except Exception:
            if attempt == 2:
                raise
    assert res is not None
    out = np.empty((4, T, D), dtype=np.float32)
    for b in range(4):
        out[b] = res.results[2 * b]["y"] + res.results[2 * b + 1]["y"]
    return out, res


def kernel(**inputs):
    out, _ = _run(inputs, trace=False)
    return out



# revision 20
# speedup vs baseline: 1.1508x; 1.0194x over previous
"""Multi-head causal attention (B=4, T=2048, D=1024, H=16, Dh=64) on 8 trn2 cores.

Sharding: 4-way DP over batch x 2-way TP over heads.
Core c handles batch c//2 and heads (c%2)*8 .. (c%2)*8+7.
Each core computes a partial output [T, D] (its heads' contribution through
w_out rows); host sums the two partials per batch.

Per-core device kernel (bf16 matmul operands, fp32 PSUM accumulation):
  v[t, f]   = sum_d xT[d, t] * w_v[d, f]      (v in [tok, feat] layout,
                                               + fused ones column per head)
  qkT[f, t] = sum_d w_qk[d, f] * xT[d, t]     (q/k in [feat, tok] layout)
  attention per (head h, q-block j of 512, group g of 2 k-tiles):
      S^T[k, q] = sum_d kT[d, k] * qT[d, q]   (only k-tiles <= q-block)
      P^T = exp(S^T / 8)                      (no max-subtraction: scores ~N(0,1))
      causal mask on diagonal groups via gpsimd affine_select (zero where k > q)
      o^T[m, q] = sum_k v_aug[k, m] * P^T[k, q]   (m: 64 v-feats + ones row
                                                   -> row 64 = softmax denominator)
      attn^T[d, q] = o^T[d, q] / o^T[64, q]   (fast recip + bf16 rank-1 PE broadcast
                                               into rows 64.. of the same bank)
  y[t, n] = sum_f attn^T[f, t] * w_o[f, n]

Scheduling: most V/QK projection groups are deferred into a filler queue and
emitted one-per-attention-group between S^T and PV so the PE always has more
queued work than ACT's exp per period -- otherwise the PE idles a few 100ns
every period, HAM re-throttles the clock to 1.2GHz, and every matmul doubles.
The softmax epilogue is similarly split into two stages popped on later
periods (the 1-lane DVE reciprocal takes ~3.4us).
"""

import numpy as np
import ml_dtypes

import concourse.mybir as mybir
import concourse.tile as tile
from concourse import bacc, bass_utils

F32 = mybir.dt.float32
BF16 = mybir.dt.bfloat16

D = 1024          # model dim
T = 2048          # tokens per batch
DH = 64           # head dim
NH_LOC = 8        # heads per core
DT = D // 128     # D tiles (contraction)
TT = T // 128     # token tiles
QB = T // 512     # q blocks of 512
VW = DH + 1       # v width incl ones column


def build_kernel():
    nc = bacc.Bacc()
    xT_d = nc.dram_tensor("xT", [D, T], BF16, kind="ExternalInput")
    wqk_d = nc.dram_tensor("w_qk", [D, 1024], BF16, kind="ExternalInput")
    wv_d = nc.dram_tensor("w_v", [D, 512], BF16, kind="ExternalInput")
    wo_d = nc.dram_tensor("w_o", [512, D], BF16, kind="ExternalInput")
    y_d = nc.dram_tensor("y", [T, D], F32, kind="ExternalOutput")

    with tile.TileContext(nc) as tc:
        with (
            tc.tile_pool(name="big", bufs=1) as big,
            tc.tile_pool(name="ptp", bufs=6) as ptp,
            tc.tile_pool(name="ovp", bufs=3) as ovp,
            tc.tile_pool(name="osb", bufs=1) as osb,
            tc.tile_pool(name="stg", bufs=2) as stg,
            tc.tile_pool(name="ps_st", bufs=2, space="PSUM") as ps_st,
            tc.tile_pool(name="ps_pv", bufs=2, space="PSUM") as ps_pv,
            tc.tile_pool(name="ps_mm", bufs=2, space="PSUM") as ps_mm,
        ):
            xt_a = big.tile([128, DT, T], BF16, tag="xt", name="xt")
            wqk_a = big.tile([128, DT, 1024], BF16, tag="wqk", name="wqk")
            wv_a = big.tile([128, DT, 512], BF16, tag="wv", name="wv")
            wo_a = big.tile([128, 4, 1024], BF16, tag="wo", name="wo")
            qk = [big.tile([128, T], BF16, tag=f"qk{i}", name=f"qk{i}") for i in range(8)]
            attn_t = [big.tile([128, T], BF16, tag=f"attn{i}", name=f"attn{i}") for i in range(4)]
            vsb_t = [big.tile([128, 2, NH_LOC * VW], BF16, tag=f"vsb{i}", name=f"vsb{i}") for i in range(8)]
            ones = big.tile([1, DH], BF16, tag="ones")
            vsb_r = [t.rearrange("p t (h c) -> p t h c", c=VW) for t in vsb_t]

            xT_r = xT_d.rearrange("(i p) t -> p i t", p=128)
            wqk_r = wqk_d.rearrange("(i p) f -> p i f", p=128)
            wv_r = wv_d.rearrange("(i p) f -> p i f", p=128)
            wo_r = wo_d.rearrange("(i p) f -> p i f", p=128)

            # wave 0 split across the three DMA-capable queues: each
            # accumulation step only waits for its own chunk
            rr = [nc.sync, nc.scalar, nc.gpsimd]
            for i in range(DT):
                rr[i % 3].dma_start(xt_a[:, i, 0:512], xT_r[:, i, 0:512])
                rr[(i + 1) % 3].dma_start(
                    wqk_a[:, i, 0:128], wqk_r[:, i, 0:128])
                rr[(i + 2) % 3].dma_start(
                    wqk_a[:, i, 512:640], wqk_r[:, i, 512:640])
            for i in range(DT):
                rr[i % 3].dma_start(wv_a[:, i, :], wv_r[:, i, :])
            rr2 = [nc.sync, nc.gpsimd]
            for tb in range(1, QB):
                for i in range(DT):
                    rr2[(tb * DT + i) % 2].dma_start(
                        xt_a[:, i, tb * 512:(tb + 1) * 512],
                        xT_r[:, i, tb * 512:(tb + 1) * 512],
                    )
            for f in (1, 5, 2, 6, 3, 7):  # in consumer (head-pair) order
                nc.sync.dma_start(
                    wqk_a[:, :, f * 128:(f + 1) * 128],
                    wqk_r[:, :, f * 128:(f + 1) * 128],
                )
            nc.sync.dma_start(wo_a, wo_r)
            nc.vector.memset(ones, 1.0)
            for t in vsb_r:
                nc.vector.memset(t[:, :, :, DH], 1.0)

            # ---- projection group emitters ----
            def v_group(tt):
                def go():
                    ps = ps_mm.tile([128, 512], F32, tag="mm")
                    for dt in range(DT):
                        nc.tensor.matmul(
                            ps,
                            lhsT=xt_a[:, dt, tt * 128:(tt + 1) * 128],
                            rhs=wv_a[:, dt, :],
                            start=(dt == 0),
                            stop=(dt == DT - 1),
                        )
                    nc.vector.tensor_copy(
                        vsb_r[tt // 2][:, tt % 2, :, 0:DH],
                        ps.rearrange("p (h c) -> p h c", c=DH),
                    )
                return go

            def qk_group(f, tb):
                def go():
                    ps = ps_mm.tile([128, 512], F32, tag="mm")
                    for dt in range(DT):
                        nc.tensor.matmul(
                            ps,
                            lhsT=wqk_a[:, dt, f * 128:(f + 1) * 128],
                            rhs=xt_a[:, dt, tb * 512:(tb + 1) * 512],
                            start=(dt == 0),
                            stop=(dt == DT - 1),
                        )
                    nc.vector.tensor_copy(qk[f][:, tb * 512:(tb + 1) * 512], ps)
                return go

            # up-front: only what attention block (pair0, j=0) needs --
            # q/k token-block 0 of head pair 0 first (attention critical
            # path), then V token tiles 0-3 (PV trails by >= 2 periods)
            qk_group(0, 0)()
            qk_group(4, 0)()
            for tt in range(4):
                v_group(tt)()

            # the rest becomes PE filler work inside the attention stream;
            # interleaved by deadline (j-block b of pair 0 needs q/k tb<=b and
            # vsb up to tile 4b+3), popped two per period while it lasts
            filler_fast = [
                v_group(4), v_group(5), qk_group(0, 1), qk_group(4, 1),
                v_group(6), v_group(7), v_group(8), v_group(9),
                qk_group(0, 2), qk_group(4, 2), v_group(10), v_group(11),
                v_group(12), v_group(13), qk_group(0, 3), qk_group(4, 3),
                v_group(14), v_group(15),
            ]
            # QK pair p must be projected before head-pair p starts (period
            # 40p); spread the groups across the preceding span so the PE
            # keeps a work surplus the whole way (HAM stays warm)
            filler_slow = []
            for p, t0, step in ((1, 13, 3), (2, 42, 4), (3, 84, 5)):
                for i, tb in enumerate(range(QB)):
                    filler_slow.append((t0 + step * (2 * i), qk_group(p, tb)))
                    filler_slow.append((t0 + step * (2 * i + 1), qk_group(4 + p, tb)))
            filler_slow.sort(key=lambda e: e[0])

            stages = []  # deferred epilogue stages (None = spacer)
            tail_reserve = []  # PE work held back to overlap the final epilogue
            period = {"i": 0}

            def period_extras():
                period["i"] += 1
                if filler_fast:
                    filler_fast.pop(0)()
                    if filler_fast:
                        filler_fast.pop(0)()
                elif filler_slow and period["i"] >= filler_slow[0][0]:
                    filler_slow.pop(0)[1]()
                if stages:
                    s = stages.pop(0)
                    if s is not None:
                        s()

            F16 = mybir.dt.float16
            out_parts = {}

            def out_partA(tt, nb):
                def go():
                    ps = ps_mm.tile([128, 512], F32, tag="mm")
                    for hp4 in range(2):
                        nc.tensor.matmul(
                            ps,
                            lhsT=attn_t[hp4][:, tt * 128:(tt + 1) * 128],
                            rhs=wo_a[:, hp4, nb * 512:(nb + 1) * 512],
                            start=(hp4 == 0),
                            stop=(hp4 == 1),
                        )
                    po = osb.tile([128, 512], F16, tag=f"osb{tt}_{nb}",
                                  name=f"osb{tt}_{nb}")
                    nc.vector.tensor_copy(po, ps)
                    out_parts[(tt, nb)] = po
                return go

            def out_partB(tt, nb):
                def go():
                    ps = ps_mm.tile([128, 512], F32, tag="mm")
                    for hp4 in (2, 3):
                        nc.tensor.matmul(
                            ps,
                            lhsT=attn_t[hp4][:, tt * 128:(tt + 1) * 128],
                            rhs=wo_a[:, hp4, nb * 512:(nb + 1) * 512],
                            start=(hp4 == 2),
                            stop=(hp4 == 3),
                        )
                    ysb = stg.tile([128, 512], F32, tag="y", bufs=4,
                                   name=f"ysb{tt}_{nb}")
                    nc.vector.tensor_add(ysb, out_parts[(tt, nb)], ps)
                    nc.sync.dma_start(
                        y_d[tt * 128:(tt + 1) * 128, nb * 512:(nb + 1) * 512],
                        ysb,
                    )
                return go

            def make_epilogue(hp, j, pvA, pvB):
                """Both heads of the pair at once: denominators into one
                [1,1024] row -> one reciprocal + one bf16 cast; two rank-1
                PE broadcasts into one [128,512] PSUM tile (head B's via
                tile_position col 64); one [128,512] multiply."""
                dn1 = stg.tile([1, 1024], F32, tag="dn", bufs=2, name=f"dn{hp}_{j}")
                rec1 = stg.tile([1, 1024], F32, tag="rec", bufs=1, name=f"rec{hp}_{j}")
                rb1 = stg.tile([1, 1024], BF16, tag="rb", name=f"rb{hp}_{j}")

                # evacuate the PV accumulator banks (incl. denominator
                # rows) immediately so the next q-block's first PV matmul
                # never stalls on the WAR
                ov = ovp.tile([128, 512], F32, tag="ov", name=f"ov{hp}_{j}")
                nc.vector.tensor_copy(dn1[0:1, 0:512], pvA[DH:DH + 1, :])
                nc.vector.tensor_copy(dn1[0:1, 512:1024], pvB[DH:DH + 1, :])
                nc.vector.tensor_copy(ov[0:DH, :], pvA[0:DH, :])
                nc.vector.tensor_copy(ov[DH:128, :], pvB[0:DH, :])

                def stage1():
                    nc.vector.reciprocal_approx_fast(out=rec1, in_=dn1)
                    nc.vector.tensor_copy(rb1, rec1)

                def stage2():
                    bc = ps_mm.tile([128, 512], F32, tag="mm")
                    nc.tensor.matmul(bc[0:DH, :], lhsT=ones,
                                     rhs=rb1[0:1, 0:512], start=True, stop=True)
                    nc.tensor.matmul(bc[DH:128, :], lhsT=ones,
                                     rhs=rb1[0:1, 512:1024], start=True,
                                     stop=True, tile_position=(0, 64))
                    nc.vector.tensor_mul(
                        attn_t[hp][:, j * 512:(j + 1) * 512], ov, bc
                    )
                return stage1, stage2

            # ---- attention: head-PAIR outer, j inner, one k-tile per period.
            # The two heads of a pair sit on partitions 0-63 / 64-127 of the
            # same qk tiles, so their K=64 S^T matmuls go to disjoint PE row
            # groups and run concurrently (weight loads overlap too).
            for hp in range(4):
                qTf = qk[hp]
                kTf = qk[4 + hp]
                for j in range(QB):
                    pvA = ps_pv.tile([128, 512], F32, tag="pv")
                    pvB = ps_pv.tile([128, 512], F32, tag="pv")
                    nkt = 4 * (j + 1)
                    pv_queue = []  # PV MMs delayed 2 periods behind S^T/exp
                    for kt in range(nkt):
                        # diagonal k-tiles: q < 128*(kt-4j) is fully masked --
                        # narrow S^T/exp/mask/PV to the live columns
                        q0 = 128 * (kt - 4 * j) if kt >= 4 * j else 0
                        nq = 512 - q0
                        st = ps_st.tile([128, 1024], F32, tag="st")
                        nc.tensor.matmul(
                            st[:, q0:512],
                            lhsT=kTf[0:64, kt * 128:(kt + 1) * 128],
                            rhs=qTf[0:64, j * 512 + q0:(j + 1) * 512],
                            start=True, stop=True,
                        )
                        nc.tensor.matmul(
                            st[:, 512 + q0:1024],
                            lhsT=kTf[64:128, kt * 128:(kt + 1) * 128],
                            rhs=qTf[64:128, j * 512 + q0:(j + 1) * 512],
                            start=True, stop=True,
                        )
                        period_extras()
                        if len(pv_queue) >= 2:
                            pv_queue.pop(0)()
                        pt = ptp.tile([128, 1024], BF16, tag="pt",
                                      name=f"pt{hp}_{j}_{kt}")
                        st_r = st.rearrange("p (h q) -> p h q", h=2)
                        pt_r = pt.rearrange("p (h q) -> p h q", h=2)
                        nc.scalar.activation(
                            pt_r[:, :, q0:512], st_r[:, :, q0:512],
                            mybir.ActivationFunctionType.Exp, scale=0.125
                        )
                        if kt >= 4 * j:  # diagonal k-tile: zero where k > q
                            # in the narrowed frame the condition is just c >= p
                            for half in range(2):
                                nc.gpsimd.affine_select(
                                    out=pt[:, half * 512 + q0:(half + 1) * 512],
                                    in_=pt[:, half * 512 + q0:(half + 1) * 512],
                                    compare_op=mybir.AluOpType.is_ge,
                                    fill=0.0,
                                    base=0,
                                    pattern=[[1, nq]],
                                    channel_multiplier=-1,
                                )

                        def pv_mms(kt=kt, pt=pt, q0=q0):
                            nc.tensor.matmul(
                                pvA[0:VW, q0:512],
                                lhsT=vsb_r[kt // 2][:, kt % 2, 2 * hp, :],
                                rhs=pt[:, q0:512],
                                start=(kt == 0), stop=(kt == nkt - 1),
                            )
                            nc.tensor.matmul(
                                pvB[0:VW, q0:512],
                                lhsT=vsb_r[kt // 2][:, kt % 2, 2 * hp + 1, :],
                                rhs=pt[:, 512 + q0:1024],
                                start=(kt == 0), stop=(kt == nkt - 1),
                            )
                        pv_queue.append(pv_mms)
                    for f_ in pv_queue:
                        f_()
                    if hp == 3 and j == QB - 1:
                        last_epi = make_epilogue(hp, j, pvA, pvB)
                    else:
                        s1, s2 = make_epilogue(hp, j, pvA, pvB)
                        stages.extend([s1, None, s2])
                    if hp == 1:
                        if j < QB - 1:
                            for tt in range(4 * j, 4 * j + 4):
                                for nb in range(2):
                                    stages.append(out_partA(tt, nb))
                        else:
                            partA_j3 = [out_partA(tt, nb)
                                        for tt in range(12, 16)
                                        for nb in range(2)]
                    if hp == 2 and j == 0:
                        stages.extend(partA_j3)
                    if hp == 3 and j < QB - 1:
                        for tt in range(4 * j, 4 * j + 4):
                            for nb in range(2):
                                if j == QB - 2 and (tt, nb) != (8, 0) and (tt, nb) != (8, 1):
                                    tail_reserve.append(out_partB(tt, nb))
                                else:
                                    stages.append(out_partB(tt, nb))

            # tail: kick the final epilogue's DVE chain first, overlap the
            # leftover stage backlog (PE work) with it, then finish the last
            # q-block's partB out-proj
            s1, s2 = last_epi
            s1()
            for fn in tail_reserve:
                fn()
            while stages:
                s = stages.pop(0)
                if s is not None:
                    s()
            s2()
            for tt in range(12, 16):
                for nb in range(2):
                    out_partB(tt, nb)()

    nc.compile()
    return nc


def _shard_inputs(x, w_qkv, w_out):
    """Build the 8 per-core input maps (matmul operands pre-cast to bf16)."""
    bf16 = ml_dtypes.bfloat16
    in_maps = []
    for c in range(8):
        b = c // 2
        hg = c % 2
        q_cols = slice(hg * 512, hg * 512 + 512)
        k_cols = slice(1024 + hg * 512, 1024 + hg * 512 + 512)
        v_cols = slice(2048 + hg * 512, 2048 + hg * 512 + 512)
        in_maps.append({
            "xT": np.ascontiguousarray(x[b].T).astype(bf16),
            "w_qk": np.ascontiguousarray(
                np.concatenate([w_qkv[:, q_cols], w_qkv[:, k_cols]], axis=1)
            ).astype(bf16),
            "w_v": np.ascontiguousarray(w_qkv[:, v_cols]).astype(bf16),
            "w_o": np.ascontiguousarray(w_out[hg * 512:hg * 512 + 512, :]).astype(bf16),
        })
    return in_maps


def _run(inputs, trace=False):
    x = np.asarray(inputs["x"], dtype=np.float32)
    w_qkv = np.asarray(inputs["w_qkv"], dtype=np.float32)
    w_out = np.asarray(inputs["w_out"], dtype=np.float32)
    nc = build_kernel()
    in_maps = _shard_inputs(x, w_qkv, w_out)
    res = None
    for attempt in range(3):
        try:
            res = bass_utils.run_bass_kernel_spmd(
                nc, in_maps, core_ids=list(range(8)), trace=trace
            )
            break
        except Exception:
            if attempt == 2:
                raise
    assert res is not None
    out = np.empty((4, T, D), dtype=np.float32)
    for b in range(4):
        out[b] = res.results[2 * b]["y"] + res.results[2 * b + 1]["y"]
    return out, res


def kernel(**inputs):
    out, _ = _run(inputs, trace=False)
    return out

